# revision 26
# baseline (speedup 1.0000x reference)
"""Self-contained Trainium2 Bass kernel for nn_BASE_6442450944602.

SE gating -> gaussian-weighted global pooling -> 3x3 patch self-attention
-> 1x1 conv + InstanceNorm + LeakyReLU(0.2).  b=16, C=512, H=W=32.

Data-parallel over batch: 8 NeuronCores x 2 samples.  One SPMD Bass/Tile
program; per-core inputs differ only in the x shard.

Layout strategy (per sample):
  - x loaded c-major [c,q] (q=h*w flat), PE-transposed to q-major [q,c].
  - out32^T = x^T * broadcast(y2)  (SE gate), bf16.
  - gaussian pool = gusT.T @ out32T on PE (bf16).
  - logits via fused DVE tensor_tensor_reduce on sigmoid(out32T) with
    partition-shifted operands; negative offsets are shifted copies of
    positive ones (L[q,-d] = L[q-d,+d]).
  - attention apply = banded-matmul: a 9-diagonal [1024,1024] matrix is
    materialized via 9 strided-diagonal DMAs into a zeroed DRAM scratch,
    band-loaded back, and PE computes g2 = AT.T @ out32T.
  - "torch view" channel reinterpretation ([q,c] -> [c',q]) done with
    partition-strided SBUF->SBUF gather DMAs while building the 1x1-conv
    rhs (cat).
  - down conv on PE; instance-norm stats on DVE/ACT; LeakyReLU fused into
    a single scalar-engine activation with per-partition scale/bias.
"""

import numpy as np
import ml_dtypes

B, C, H, W = 16, 512, 32, 32
HW = H * W
NCORES = 8
BS = B // NCORES  # samples per core

_CACHE = {}


def _build_nc():
    import concourse.bacc as bacc
    import concourse.bass as bass
    import concourse.mybir as mybir
    import concourse.tile as tile
    from concourse.bass_types import AP

    f32 = mybir.dt.float32
    bf16 = mybir.dt.bfloat16
    FT = mybir.ActivationFunctionType
    ALU = mybir.AluOpType
    AX = mybir.AxisListType

    nc = bacc.Bacc("TRN2", target_bir_lowering=False, debug=False)

    x2 = nc.dram_tensor("x2", [BS, C, HW], f32, kind="ExternalInput").ap()
    gusT_d = nc.dram_tensor("gusT", [HW, HW], bf16, kind="ExternalInput").ap()
    dw1T_d = nc.dram_tensor("dw1T", [C, C], bf16, kind="ExternalInput").ap()
    d2eT_d = nc.dram_tensor("d2eT", [HW, C], bf16, kind="ExternalInput").ap()
    d2oT_d = nc.dram_tensor("d2oT", [HW, C], bf16, kind="ExternalInput").ap()
    w1T_d = nc.dram_tensor("w1T", [C, 32], f32, kind="ExternalInput").ap()
    b1_d = nc.dram_tensor("b1", [32, 1], f32, kind="ExternalInput").ap()
    w2T_d = nc.dram_tensor("w2T", [32, C], f32, kind="ExternalInput").ap()
    b2_d = nc.dram_tensor("b2", [C, 1], f32, kind="ExternalInput").ap()
    ident_d = nc.dram_tensor("ident", [128, 128], f32, kind="ExternalInput").ap()
    ones_d = nc.dram_tensor("ones1", [1, 128], f32, kind="ExternalInput").ap()
    oneh_d = nc.dram_tensor("oneh", [9, 128, 9], bf16, kind="ExternalInput").ap()
    maskL_d = nc.dram_tensor("maskL", [8, 128, 9], f32, kind="ExternalInput").ap()
    maskA_d = nc.dram_tensor("maskA", [128, 9], bf16, kind="ExternalInput").ap()
    atz_d = nc.dram_tensor("atz", [BS, HW, HW], bf16, kind="ExternalInput").ap()
    attn_d = nc.dram_tensor("attn_d", [BS, HW, 9], bf16, kind="Internal").ap()
    out2 = nc.dram_tensor("out2", [BS, C, HW], f32, kind="ExternalOutput").ap()

    # positive patch offsets (delta, logits column); negatives are copies
    POSD = [(0, 4), (1, 5), (31, 6), (32, 7), (33, 8)]
    ALLD = [(-33, 0), (-32, 1), (-31, 2), (-1, 3), (0, 4),
            (1, 5), (31, 6), (32, 7), (33, 8)]

    with tile.TileContext(nc) as tc:
        from contextlib import ExitStack
        with ExitStack() as ctx:
            cst = ctx.enter_context(tc.tile_pool(name="cst", bufs=1))
            xp = ctx.enter_context(tc.tile_pool(name="xp", bufs=5))
            otp = ctx.enter_context(tc.tile_pool(name="otp", bufs=12))
            sgc = ctx.enter_context(tc.tile_pool(name="sgc", bufs=5))
            g2p = ctx.enter_context(tc.tile_pool(name="g2p", bufs=12))
            catp = ctx.enter_context(tc.tile_pool(name="catp", bufs=8))
            atp = ctx.enter_context(tc.tile_pool(name="atp", bufs=8))
            ltp = ctx.enter_context(tc.tile_pool(name="ltp", bufs=16))
            anp = ctx.enter_context(tc.tile_pool(name="anp", bufs=8))
            scrp = ctx.enter_context(tc.tile_pool(name="scrp", bufs=6))
            zop = ctx.enter_context(tc.tile_pool(name="zop", bufs=4))
            sqp = ctx.enter_context(tc.tile_pool(name="sqp", bufs=2))
            y2p = ctx.enter_context(tc.tile_pool(name="y2p", bufs=2))
            smp = ctx.enter_context(tc.tile_pool(name="smp", bufs=24))
            # PSUM pools (8 banks): pts 2 (x transposes), pg 2 (gus+g2
            # chains), pz 2 (z / SE / logit transposes), pl 2 ([9,1024])
            pts = ctx.enter_context(
                tc.tile_pool(name="pts", bufs=2, space="PSUM"))
            pg = ctx.enter_context(
                tc.tile_pool(name="pg", bufs=2, space="PSUM"))
            pz = ctx.enter_context(
                tc.tile_pool(name="pz", bufs=2, space="PSUM"))
            pl = ctx.enter_context(
                tc.tile_pool(name="pl", bufs=1, space="PSUM"))

            # ---- constants ----
            gus_sb = []
            for t in range(8):
                g = cst.tile([128, HW], bf16, name=f"gus_sb{t}", tag=f"gus{t}")
                nc.sync.dma_start(g[:], gusT_d[128 * t:128 * (t + 1), :])
                gus_sb.append(g)
            dw1_sb = []
            for t in range(4):
                d = cst.tile([128, C], bf16, name=f"dw1_sb{t}", tag=f"dw1{t}")
                nc.sync.dma_start(d[:], dw1T_d[128 * t:128 * (t + 1), :])
                dw1_sb.append(d)
            d2e_sb, d2o_sb = [], []
            for t in range(8):
                de = cst.tile([128, C], bf16, name=f"d2e_sb{t}", tag=f"d2e{t}")
                nc.sync.dma_start(de[:], d2eT_d[128 * t:128 * (t + 1), :])
                d2e_sb.append(de)
                do = cst.tile([128, C], bf16, name=f"d2o_sb{t}", tag=f"d2o{t}")
                nc.sync.dma_start(do[:], d2oT_d[128 * t:128 * (t + 1), :])
                d2o_sb.append(do)
            w1_sb = []
            for t in range(4):
                wt = cst.tile([128, 32], f32, name=f"w1_sb{t}", tag=f"w1{t}")
                nc.sync.dma_start(wt[:], w1T_d[128 * t:128 * (t + 1), :])
                w1_sb.append(wt)
            w2_sb = cst.tile([32, C], f32, name="w2_sb", tag="w2")
            nc.sync.dma_start(w2_sb[:], w2T_d[:])
            b1_sb = cst.tile([32, 1], f32, name="b1_sb", tag="b1")
            nc.sync.dma_start(b1_sb[:], b1_d[:])
            b2_sb = []
            for t in range(4):
                bt = cst.tile([128, 1], f32, name=f"b2_sb{t}", tag=f"b2{t}")
                nc.sync.dma_start(bt[:], b2_d[128 * t:128 * (t + 1), :])
                b2_sb.append(bt)
            ident_sb = cst.tile([128, 128], f32, name="ident_sb", tag="id")
            nc.sync.dma_start(ident_sb[:], ident_d[:])
            ones_sb = cst.tile([1, 128], f32, name="ones_sb", tag="on")
            nc.sync.dma_start(ones_sb[:], ones_d[:])
            oneh_sb = []
            for k in range(9):
                oh = cst.tile([128, 9], bf16, name=f"oneh_sb{k}", tag=f"oh{k}")
                nc.sync.dma_start(oh[:], oneh_d[k, :, :])
                oneh_sb.append(oh)
            maskL_sb = []
            for t in range(8):
                m = cst.tile([128, 9], f32, name=f"maskL_sb{t}", tag=f"mL{t}")
                nc.sync.dma_start(m[:], maskL_d[t, :, :])
                maskL_sb.append(m)
            maskA_sb = cst.tile([128, 9], bf16, name="maskA_sb", tag="mA")
            nc.sync.dma_start(maskA_sb[:], maskA_d[:])
            eps_sb = cst.tile([128, 1], f32, name="eps_sb", tag="eps")
            nc.vector.memset(eps_sb[:], 1e-5)

            for s in range(BS):
                # ---- load x (c-major) + spatial-sum for SE ----
                Xs = []
                for ct in range(4):
                    Xt = xp.tile([128, HW], f32, name=f"X{s}_{ct}", tag="x")
                    nc.sync.dma_start(
                        Xt[:], x2[s, 128 * ct:128 * (ct + 1), :])
                    Xs.append(Xt)
                svs = []
                for ct in range(4):
                    sv = smp.tile([128, 1], f32, name=f"sv{s}_{ct}", tag="sm")
                    nc.vector.tensor_reduce(sv[:], Xs[ct][:], AX.X, ALU.add)
                    svs.append(sv)
                # ---- SE: y1 = relu(w1 @ s/HW + b1) ----
                p_y1 = pz.tile([32, 1], f32, name=f"py1{s}", tag="t")
                for ct in range(4):
                    nc.tensor.matmul(p_y1[:], w1_sb[ct][:], svs[ct][:],
                                     start=(ct == 0), stop=(ct == 3))
                y1 = smp.tile([32, 1], f32, name=f"y1{s}", tag="sm")
                nc.scalar.activation(y1[:], p_y1[:], FT.Relu, bias=b1_sb[:])
                # ---- SE: y2 = sigmoid(w2 @ y1 + b2) ----
                y2cols = []
                for ct in range(4):
                    p_y2 = pz.tile([128, 1], f32, name=f"py2{s}_{ct}", tag="t")
                    nc.tensor.matmul(p_y2[:], w2_sb[:, 128 * ct:128 * (ct + 1)],
                                     y1[:], start=True, stop=True)
                    y2c = smp.tile([128, 1], f32, name=f"y2c{s}_{ct}", tag="sm")
                    nc.scalar.activation(y2c[:], p_y2[:], FT.Sigmoid,
                                         bias=b2_sb[ct][:])
                    y2cols.append(y2c)
                # y2 row [1, C] then broadcast to [128, C]
                p_row = pz.tile([1, C], f32, name=f"prow{s}", tag="t")
                for ct in range(4):
                    nc.tensor.matmul(p_row[:, 128 * ct:128 * (ct + 1)],
                                     y2cols[ct][:], ident_sb[:],
                                     start=True, stop=True)
                y2row = smp.tile([1, C], f32, name=f"y2row{s}", tag="y2r", bufs=2)
                nc.scalar.copy(y2row[:], p_row[:])
                p_y2b = pz.tile([128, C], f32, name=f"py2b{s}", tag="t")
                nc.tensor.matmul(p_y2b[:], ones_sb[:], y2row[:],
                                 start=True, stop=True)
                y2b = y2p.tile([128, C], f32, name=f"y2b{s}", tag="t")
                nc.scalar.copy(y2b[:], p_y2b[:])

                # ---- gated activations, c-major: sig_c = sigmoid(x*y2) ----
                SCs = []
                for ct in range(4):
                    oc = sgc.tile([128, HW], bf16, name=f"oc{s}_{ct}", tag="oc")
                    nc.vector.tensor_scalar_mul(oc[:], Xs[ct][:],
                                                y2cols[ct][:])
                    sc = sgc.tile([128, HW], bf16, name=f"sc{s}_{ct}", tag="sc")
                    nc.scalar.activation(sc[:], oc[:], FT.Sigmoid)
                    SCs.append(sc)

                # ---- transpose x, apply gate (q-major) ----
                OTs = []
                for tk in range(8):
                    p_xt = pts.tile([128, C], f32, name=f"pxt{s}_{tk}", tag="t")
                    for ct in range(4):
                        nc.tensor.transpose(
                            p_xt[:, 128 * ct:128 * (ct + 1)],
                            Xs[ct][:, 128 * tk:128 * (tk + 1)],
                            ident_sb[:])
                    OT = otp.tile([128, C], bf16, name=f"OT{s}_{tk}", tag="t")
                    nc.vector.tensor_tensor(OT[:], p_xt[:], y2b[:], ALU.mult)
                    OTs.append(OT)

                # ---- cat rhs tiles for the gaussian half ([c', q] view) ----
                CATs = []
                for tcc in range(4):
                    cat_t = catp.tile([128, HW], bf16, name=f"CAT{s}_{tcc}",
                                      tag="t")
                    CATs.append(cat_t)

                # ---- gaussian pooling on PE ----
                # gusT columns are host-permuted (evens first) so psum
                # partitions [0:64) are even p (first q-half of channel
                # p/2) and [64:128) odd p (second half).
                for tm in range(8):
                    p_g = pg.tile([128, C], f32, name=f"pgus{s}_{tm}", tag="t")
                    for tk in range(8):
                        nc.tensor.matmul(
                            p_g[:], gus_sb[tk][:, 128 * tm:128 * (tm + 1)],
                            OTs[tk][:], start=(tk == 0), stop=(tk == 7))
                    base = (tm % 2) * 64
                    dst = CATs[tm // 2]
                    nc.scalar.copy(dst[base:base + 64, 0:C], p_g[0:64, :])
                    nc.scalar.copy(dst[base:base + 64, C:2 * C],
                                   p_g[64:128, :])

                # ---- patch logits: c-major shifted products, one-hot PE
                # reduce over channels into psum [9, 1024] (the 1/C scale is
                # folded into the one-hot weights) ----
                PAD = 64
                PW = PAD + HW + 64
                p_L = pl.tile([9, HW], f32, name=f"pL{s}", tag="t")
                n_mm = {0: 9 * 4, 1: 9 * 4}
                i_mm = {0: 0, 1: 0}
                for ct in range(4):
                    Pd = {}
                    for dpos in (0, 1, 31, 32, 33):
                        P = scrp.tile([128, PW], bf16,
                                      name=f"P{s}_{ct}_{dpos}", tag="t")
                        nc.vector.memset(P[:, 0:PAD], 0.0)
                        nc.vector.memset(P[:, PAD + HW - dpos:PW], 0.0)
                        nc.vector.tensor_tensor(
                            P[:, PAD:PAD + HW - dpos],
                            SCs[ct][:, 0:HW - dpos],
                            SCs[ct][:, dpos:HW], ALU.mult)
                        Pd[dpos] = P
                    for hh in range(2):
                        for d, k in ALLD:
                            P = Pd[abs(d)]
                            off = PAD + C * hh - (abs(d) if d < 0 else 0)
                            nc.tensor.matmul(
                                p_L[0:9, C * hh:C * (hh + 1)],
                                oneh_sb[k][:], P[:, off:off + C],
                                start=(i_mm[hh] == 0),
                                stop=(i_mm[hh] == n_mm[hh] - 1))
                            i_mm[hh] += 1
                L_c = ltp.tile([9, HW], f32, name=f"Lc{s}", tag="lc", bufs=2)
                nc.scalar.copy(L_c[0:9, 0:C], p_L[0:9, 0:C])
                nc.scalar.copy(L_c[0:9, C:HW], p_L[0:9, C:HW])

                # ---- transpose logits to q-major, mask, softmax, export ----
                for t in range(8):
                    p_lt = pz.tile([128, 9], f32, name=f"plt{s}_{t}", tag="t")
                    nc.tensor.transpose(p_lt[:],
                                        L_c[0:9, 128 * t:128 * (t + 1)],
                                        ident_sb[0:9, 0:9])
                    Lm = ltp.tile([128, 9], f32, name=f"Lm{s}_{t}", tag="t")
                    nc.vector.tensor_tensor(Lm[:], p_lt[:], maskL_sb[t][:],
                                            ALU.mult)
                    nmax = smp.tile([128, 1], f32, name=f"nm{s}_{t}", tag="sm")
                    nc.vector.tensor_reduce(nmax[:], Lm[:], AX.X, ALU.max,
                                            negate=True)
                    ee = ltp.tile([128, 9], f32, name=f"e{s}_{t}", tag="e")
                    ssum = smp.tile([128, 1], f32, name=f"ssum{s}_{t}",
                                    tag="sm")
                    nc.scalar.activation(ee[:], Lm[:], FT.Exp,
                                         bias=nmax[:], accum_out=ssum[:])
                    rin = smp.tile([128, 1], f32, name=f"rin{s}_{t}", tag="sm")
                    nc.vector.reciprocal(rin[:], ssum[:])
                    atn = anp.tile([128, 9], bf16, name=f"atn{s}_{t}", tag="t")
                    nc.vector.tensor_scalar(atn[:], ee[:], rin[:], None,
                                            ALU.mult)
                    nc.vector.tensor_mul(atn[:], atn[:], maskA_sb[:])
                    nc.sync.dma_start(attn_d[s, 128 * t:128 * (t + 1), :],
                                      atn[:])

                # ---- build banded attention matrix via diagonal DMAs ----
                at_tensor = atz_d.tensor
                an_tensor = attn_d.tensor
                with nc.allow_non_contiguous_dma(reason="diagonal scatter"):
                    for d, k in ALLD:
                        cnt = HW - abs(d)
                        q0 = max(0, -d)
                        src = AP(tensor=an_tensor,
                                 offset=s * HW * 9 + q0 * 9 + k,
                                 ap=[[9, cnt], [1, 1]])
                        dst = AP(tensor=at_tensor,
                                 offset=s * HW * HW + (q0 + d) * HW + q0,
                                 ap=[[HW + 1, cnt], [1, 1]])
                        nc.sync.dma_start(dst, src)
                # band loads
                ATs = []
                for tk in range(8):
                    c0 = 128 * max(tk - 1, 0)
                    c1 = 128 * min(tk + 2, 8)
                    wdt = c1 - c0
                    atb = atp.tile([128, 384], bf16, name=f"atb{s}_{tk}",
                                   tag="t")
                    src = AP(tensor=at_tensor,
                             offset=s * HW * HW + 128 * tk * HW + c0,
                             ap=[[HW, 128], [1, wdt]])
                    nc.sync.dma_start(atb[:, 0:wdt], src)
                    ATs.append((atb, c0))

                # ---- attention apply: g2 = AT.T @ OT on PE ----
                G2s = []
                for tm in range(8):
                    p_a = pg.tile([128, C], f32, name=f"pat{s}_{tm}", tag="t")
                    ks = [t for t in (tm - 1, tm, tm + 1) if 0 <= t < 8]
                    for i, tk in enumerate(ks):
                        atb, c0 = ATs[tk]
                        coff = 128 * tm - c0
                        nc.tensor.matmul(
                            p_a[:], atb[:, coff:coff + 128], OTs[tk][:],
                            start=(i == 0), stop=(i == len(ks) - 1))
                    g2b = g2p.tile([128, C], bf16, name=f"g2b{s}_{tm}", tag="t")
                    nc.scalar.copy(g2b[:], p_a[:])
                    G2s.append(g2b)

                # ---- down conv + instance norm + leaky relu ----
                # csa half uses zero-interleaved expanded weights (d2e/d2o)
                # against g2 [q, c] directly -- the channel-pair view is
                # folded into the weights.
                for tm in range(4):
                    zparts = []
                    for hh in range(2):
                        p_z = pz.tile([128, C], f32, name=f"pz{s}_{tm}_{hh}",
                                      tag="t")
                        d2_sb = d2e_sb if hh == 0 else d2o_sb
                        for tk in range(4):
                            nc.tensor.matmul(
                                p_z[:],
                                dw1_sb[tk][:, 128 * tm:128 * (tm + 1)],
                                CATs[tk][:, C * hh:C * (hh + 1)],
                                start=(tk == 0), stop=False)
                        for tk in range(8):
                            nc.tensor.matmul(
                                p_z[:],
                                d2_sb[tk][:, 128 * tm:128 * (tm + 1)],
                                G2s[tk][:],
                                start=False, stop=(tk == 7))
                        zparts.append(p_z)
                    r0 = smp.tile([128, 1], f32, name=f"r0{s}_{tm}", tag="sm")
                    r1 = smp.tile([128, 1], f32, name=f"r1{s}_{tm}", tag="sm")
                    nc.vector.tensor_reduce(r0[:], zparts[0][:], AX.X, ALU.add)
                    nc.vector.tensor_reduce(r1[:], zparts[1][:], AX.X, ALU.add)
                    msum = smp.tile([128, 1], f32, name=f"ms{s}_{tm}", tag="sm")
                    nc.vector.tensor_add(msum[:], r0[:], r1[:])
                    sq0 = smp.tile([128, 1], f32, name=f"sq0{s}_{tm}", tag="sm")
                    sq1 = smp.tile([128, 1], f32, name=f"sq1{s}_{tm}", tag="sm")
                    sqs0 = sqp.tile([128, C], f32, name=f"sqs0{s}_{tm}",
                                    tag="t")
                    sqs1 = sqp.tile([128, C], f32, name=f"sqs1{s}_{tm}",
                                    tag="t")
                    nc.scalar.activation(sqs0[:], zparts[0][:], FT.Square,
                                         accum_out=sq0[:])
                    nc.scalar.activation(sqs1[:], zparts[1][:], FT.Square,
                                         accum_out=sq1[:])
                    sqsum = smp.tile([128, 1], f32, name=f"sqm{s}_{tm}",
                                     tag="sm")
                    nc.vector.tensor_add(sqsum[:], sq0[:], sq1[:])
                    m1 = smp.tile([128, 1], f32, name=f"m1{s}_{tm}", tag="sm")
                    nc.vector.tensor_scalar_mul(m1[:], msum[:], 1.0 / HW)
                    m2 = smp.tile([128, 1], f32, name=f"m2{s}_{tm}", tag="sm")
                    nc.vector.tensor_scalar_mul(m2[:], sqsum[:], 1.0 / HW)
                    m1sq = smp.tile([128, 1], f32, name=f"m1s{s}_{tm}",
                                    tag="sm")
                    nc.vector.tensor_mul(m1sq[:], m1[:], m1[:])
                    var = smp.tile([128, 1], f32, name=f"var{s}_{tm}", tag="sm")
                    nc.vector.tensor_sub(var[:], m2[:], m1sq[:])
                    sdev = smp.tile([128, 1], f32, name=f"sd{s}_{tm}", tag="sm")
                    nc.scalar.activation(sdev[:], var[:], FT.Sqrt,
                                         bias=eps_sb[:])
                    rstd = smp.tile([128, 1], f32, name=f"rs{s}_{tm}", tag="sm")
                    nc.vector.reciprocal(rstd[:], sdev[:])
                    nmr = smp.tile([128, 1], f32, name=f"nmr{s}_{tm}", tag="sm")
                    nc.vector.tensor_scalar(nmr[:], m1[:], rstd[:], -1.0,
                                            ALU.mult, ALU.mult)
                    for hh in range(2):
                        # zn = z*rstd - mu*rstd, then leaky = max(zn, 0.2*zn)
                        zn = sqp.tile([128, C], f32, name=f"zn{s}_{tm}_{hh}",
                                      tag="zn")
                        nc.scalar.activation(zn[:], zparts[hh][:], FT.Identity,
                                             bias=nmr[:], scale=rstd[:])
                        zs = sqp.tile([128, C], f32, name=f"zs{s}_{tm}_{hh}",
                                      tag="zs")
                        nc.vector.tensor_scalar_mul(zs[:], zn[:], 0.2)
                        zfin = zop.tile([128, C], f32, name=f"zf{s}_{tm}_{hh}",
                                        tag="t")
                        nc.vector.tensor_tensor(zfin[:], zn[:], zs[:], ALU.max)
                        nc.sync.dma_start(
                            out2[s, 128 * tm:128 * (tm + 1),
                                 C * hh:C * (hh + 1)],
                            zfin[:])

    nc.compile()
    return nc


def _host_prep(x, se_w1, se_b1, se_w2, se_b2, down_w, gus):
    bf = ml_dtypes.bfloat16
    x = np.ascontiguousarray(np.asarray(x, np.float32).reshape(B, C, HW))
    gusT = np.ascontiguousarray(np.asarray(gus, np.float32).reshape(HW, HW).T)
    # permute columns within each 128-block: evens first, then odds, so the
    # gaussian matmul's psum partitions give contiguous channel-halves
    perm = np.concatenate([np.arange(0, 128, 2), np.arange(1, 128, 2)])
    gusT = gusT.reshape(HW, 8, 128)[:, :, perm].reshape(HW, HW).astype(bf)
    dw = np.asarray(down_w, np.float32)
    dw1T = np.ascontiguousarray(dw[:, :C].T).astype(bf)           # [C, C]
    dw2 = dw[:, C:]                                               # [C, C]
    d2eT = np.zeros((HW, C), np.float32)
    d2oT = np.zeros((HW, C), np.float32)
    d2eT[0::2, :] = dw2.T
    d2oT[1::2, :] = dw2.T
    d2eT = d2eT.astype(bf)
    d2oT = d2oT.astype(bf)
    w1T = np.ascontiguousarray(np.asarray(se_w1, np.float32).T) / HW
    w2T = np.ascontiguousarray(np.asarray(se_w2, np.float32).T)
    b1 = np.asarray(se_b1, np.float32).reshape(32, 1)
    b2 = np.asarray(se_b2, np.float32).reshape(C, 1)
    ident = np.eye(128, dtype=np.float32)
    ones1 = np.ones((1, 128), np.float32)
    oneh = np.zeros((9, 128, 9), np.float32)
    for k in range(9):
        oneh[k, :, k] = 1.0 / C
    oneh = oneh.astype(bf)

    # masks over the logits tiles: [8 tiles, 128 rows, 9 offsets]
    # x-edges: x==0 kills dx=-1 cols {0,3,6}; x==31 kills dx=+1 cols {2,5,8}
    r = np.arange(128)
    maskA_x = np.ones((128, 9), np.float32)
    maskA_x[r % 32 == 0, 0] = 0
    maskA_x[r % 32 == 0, 3] = 0
    maskA_x[r % 32 == 0, 6] = 0
    maskA_x[r % 32 == 31, 2] = 0
    maskA_x[r % 32 == 31, 5] = 0
    maskA_x[r % 32 == 31, 8] = 0
    maskL = np.broadcast_to(maskA_x, (8, 128, 9)).copy()
    # y-edges: first image row (tile 0, rows<32) kills dy=-1 cols {0,1,2};
    # last image row (tile 7, rows>=96) kills dy=+1 cols {6,7,8}
    maskL[0, :32, 0:3] = 0
    maskL[7, 96:, 6:9] = 0
    # attention export mask: x-edges only; y-edge entries are excluded by
    # the diagonal DMA ranges.
    maskA_t = maskA_x.astype(bf)

    atz = np.zeros((BS, HW, HW), bf)

    shared = {
        "gusT": gusT, "dw1T": dw1T, "d2eT": d2eT, "d2oT": d2oT,
        "w1T": w1T, "w2T": w2T,
        "b1": b1, "b2": b2, "ident": ident, "ones1": ones1, "oneh": oneh,
        "maskL": maskL, "maskA": maskA_t, "atz": atz,
    }
    in_maps = []
    for i in range(NCORES):
        m = dict(shared)
        m["x2"] = np.ascontiguousarray(x[BS * i:BS * (i + 1)])
        in_maps.append(m)
    return in_maps


def _ensure_ntff_hook():
    """Wire the missing antenv.axon_hooks shim so trace=True works."""
    import sys
    import types
    try:
        import antenv.axon_hooks  # noqa: F401
        return
    except ImportError:
        pass
    from trn_agent_boot.trn_boot import _ntff_profile_via_ctypes
    hook = _ntff_profile_via_ctypes("/opt/axon/libaxon_pjrt.so")
    mod = types.ModuleType("antenv.axon_hooks")
    mod.get_axon_ntff_profile_hook = lambda: hook
    mod.set_axon_ntff_profile_hook = lambda h: None
    sys.modules["antenv.axon_hooks"] = mod


def kernel(x, se_w1, se_b1, se_w2, se_b2, down_w, gus, _trace=False):
    from concourse import bass_utils

    if _trace:
        try:
            _ensure_ntff_hook()
        except Exception:
            _trace = False
    if "nc" not in _CACHE:
        _CACHE["nc"] = _build_nc()
    nc = _CACHE["nc"]
    in_maps = _host_prep(x, se_w1, se_b1, se_w2, se_b2, down_w, gus)
    res = bass_utils.run_bass_kernel_spmd(
        nc, in_maps, core_ids=list(range(NCORES)), trace=_trace)
    _CACHE["last_result"] = res
    outs = [res.results[i]["out2"] for i in range(NCORES)]
    out = np.concatenate(outs, axis=0).reshape(B, C, H, W)
    return out.astype(np.float32)


# revision 27
# speedup vs baseline: 1.0515x; 1.0515x over previous
"""Self-contained Trainium2 Bass kernel for nn_BASE_6442450944602.

SE gating -> gaussian-weighted global pooling -> 3x3 patch self-attention
-> 1x1 conv + InstanceNorm + LeakyReLU(0.2).  b=16, C=512, H=W=32.

Data-parallel over batch: 8 NeuronCores x 2 samples.  One SPMD Bass/Tile
program; per-core inputs differ only in the x shard.

Layout strategy (per sample):
  - x loaded c-major [c,q] (q=h*w flat), PE-transposed to q-major [q,c].
  - out32^T = x^T * broadcast(y2)  (SE gate), bf16.
  - gaussian pool = gusT.T @ out32T on PE (bf16).
  - logits via fused DVE tensor_tensor_reduce on sigmoid(out32T) with
    partition-shifted operands; negative offsets are shifted copies of
    positive ones (L[q,-d] = L[q-d,+d]).
  - attention apply = banded-matmul: a 9-diagonal [1024,1024] matrix is
    materialized via 9 strided-diagonal DMAs into a zeroed DRAM scratch,
    band-loaded back, and PE computes g2 = AT.T @ out32T.
  - "torch view" channel reinterpretation ([q,c] -> [c',q]) done with
    partition-strided SBUF->SBUF gather DMAs while building the 1x1-conv
    rhs (cat).
  - down conv on PE; instance-norm stats on DVE/ACT; LeakyReLU fused into
    a single scalar-engine activation with per-partition scale/bias.
"""

import numpy as np
import ml_dtypes

B, C, H, W = 16, 512, 32, 32
HW = H * W
NCORES = 8
BS = B // NCORES  # samples per core

_CACHE = {}


def _build_nc():
    import concourse.bacc as bacc
    import concourse.bass as bass
    import concourse.mybir as mybir
    import concourse.tile as tile
    from concourse.bass_types import AP

    f32 = mybir.dt.float32
    bf16 = mybir.dt.bfloat16
    FT = mybir.ActivationFunctionType
    ALU = mybir.AluOpType
    AX = mybir.AxisListType

    nc = bacc.Bacc("TRN2", target_bir_lowering=False, debug=False)

    x2 = nc.dram_tensor("x2", [BS, C, HW], f32, kind="ExternalInput").ap()
    gusT_d = nc.dram_tensor("gusT", [HW, HW], bf16, kind="ExternalInput").ap()
    dw1T_d = nc.dram_tensor("dw1T", [C, C], bf16, kind="ExternalInput").ap()
    d2eT_d = nc.dram_tensor("d2eT", [HW, C], bf16, kind="ExternalInput").ap()
    d2oT_d = nc.dram_tensor("d2oT", [HW, C], bf16, kind="ExternalInput").ap()
    w1T_d = nc.dram_tensor("w1T", [C, 32], f32, kind="ExternalInput").ap()
    b1_d = nc.dram_tensor("b1", [32, 1], f32, kind="ExternalInput").ap()
    w2T_d = nc.dram_tensor("w2T", [32, C], f32, kind="ExternalInput").ap()
    b2_d = nc.dram_tensor("b2", [C, 1], f32, kind="ExternalInput").ap()
    ident_d = nc.dram_tensor("ident", [128, 128], f32, kind="ExternalInput").ap()
    ones_d = nc.dram_tensor("ones1", [1, 128], f32, kind="ExternalInput").ap()
    oneh_d = nc.dram_tensor("oneh", [9, 128, 9], bf16, kind="ExternalInput").ap()
    maskL_d = nc.dram_tensor("maskL", [8, 128, 9], f32, kind="ExternalInput").ap()
    maskA_d = nc.dram_tensor("maskA", [128, 9], bf16, kind="ExternalInput").ap()
    atz_d = nc.dram_tensor("atz", [BS, HW, HW], bf16, kind="ExternalInput").ap()
    attn_d = nc.dram_tensor("attn_d", [BS, HW, 9], bf16, kind="Internal").ap()
    out2 = nc.dram_tensor("out2", [BS, C, HW], f32, kind="ExternalOutput").ap()

    # positive patch offsets (delta, logits column); negatives are copies
    POSD = [(0, 4), (1, 5), (31, 6), (32, 7), (33, 8)]
    ALLD = [(-33, 0), (-32, 1), (-31, 2), (-1, 3), (0, 4),
            (1, 5), (31, 6), (32, 7), (33, 8)]

    with tile.TileContext(nc) as tc:
        from contextlib import ExitStack
        with ExitStack() as ctx:
            cst = ctx.enter_context(tc.tile_pool(name="cst", bufs=1))
            xp = ctx.enter_context(tc.tile_pool(name="xp", bufs=8))
            otp = ctx.enter_context(tc.tile_pool(name="otp", bufs=10))
            sgc = ctx.enter_context(tc.tile_pool(name="sgc", bufs=5))
            g2p = ctx.enter_context(tc.tile_pool(name="g2p", bufs=10))
            catp = ctx.enter_context(tc.tile_pool(name="catp", bufs=8))
            atp = ctx.enter_context(tc.tile_pool(name="atp", bufs=8))
            ltp = ctx.enter_context(tc.tile_pool(name="ltp", bufs=16))
            anp = ctx.enter_context(tc.tile_pool(name="anp", bufs=8))
            scrp = ctx.enter_context(tc.tile_pool(name="scrp", bufs=6))
            zop = ctx.enter_context(tc.tile_pool(name="zop", bufs=4))
            sqp = ctx.enter_context(tc.tile_pool(name="sqp", bufs=2))
            y2p = ctx.enter_context(tc.tile_pool(name="y2p", bufs=2))
            smp = ctx.enter_context(tc.tile_pool(name="smp", bufs=24))
            # PSUM pools (8 banks): pts 2 (x transposes), pg 2 (gus+g2
            # chains), pz 2 (z / SE / logit transposes), pl 2 ([9,1024])
            pts = ctx.enter_context(
                tc.tile_pool(name="pts", bufs=2, space="PSUM"))
            pg = ctx.enter_context(
                tc.tile_pool(name="pg", bufs=2, space="PSUM"))
            pz = ctx.enter_context(
                tc.tile_pool(name="pz", bufs=2, space="PSUM"))
            pl = ctx.enter_context(
                tc.tile_pool(name="pl", bufs=1, space="PSUM"))

            # ---- constants ----
            big_loads = []
            gus_sb = []
            for t in range(8):
                g = cst.tile([128, HW], bf16, name=f"gus_sb{t}", tag=f"gus{t}")
                big_loads.append((g, gusT_d[128 * t:128 * (t + 1), :]))
                gus_sb.append(g)
            dw1_sb = []
            for t in range(4):
                d = cst.tile([128, C], bf16, name=f"dw1_sb{t}", tag=f"dw1{t}")
                big_loads.append((d, dw1T_d[128 * t:128 * (t + 1), :]))
                dw1_sb.append(d)
            d2e_sb, d2o_sb = [], []
            for t in range(8):
                de = cst.tile([128, C], bf16, name=f"d2e_sb{t}", tag=f"d2e{t}")
                big_loads.append((de, d2eT_d[128 * t:128 * (t + 1), :]))
                d2e_sb.append(de)
                do = cst.tile([128, C], bf16, name=f"d2o_sb{t}", tag=f"d2o{t}")
                big_loads.append((do, d2oT_d[128 * t:128 * (t + 1), :]))
                d2o_sb.append(do)
            w1_sb = []
            for t in range(4):
                wt = cst.tile([128, 32], f32, name=f"w1_sb{t}", tag=f"w1{t}")
                nc.sync.dma_start(wt[:], w1T_d[128 * t:128 * (t + 1), :])
                w1_sb.append(wt)
            w2_sb = cst.tile([32, C], f32, name="w2_sb", tag="w2")
            nc.sync.dma_start(w2_sb[:], w2T_d[:])
            b1_sb = cst.tile([32, 1], f32, name="b1_sb", tag="b1")
            nc.sync.dma_start(b1_sb[:], b1_d[:])
            b2_sb = []
            for t in range(4):
                bt = cst.tile([128, 1], f32, name=f"b2_sb{t}", tag=f"b2{t}")
                nc.sync.dma_start(bt[:], b2_d[128 * t:128 * (t + 1), :])
                b2_sb.append(bt)
            ident_sb = cst.tile([128, 128], f32, name="ident_sb", tag="id")
            nc.sync.dma_start(ident_sb[:], ident_d[:])
            ones_sb = cst.tile([1, 128], f32, name="ones_sb", tag="on")
            nc.sync.dma_start(ones_sb[:], ones_d[:])
            oneh_sb = []
            for k in range(9):
                oh = cst.tile([128, 9], bf16, name=f"oneh_sb{k}", tag=f"oh{k}")
                nc.sync.dma_start(oh[:], oneh_d[k, :, :])
                oneh_sb.append(oh)
            maskL_sb = []
            for t in range(8):
                m = cst.tile([128, 9], f32, name=f"maskL_sb{t}", tag=f"mL{t}")
                nc.sync.dma_start(m[:], maskL_d[t, :, :])
                maskL_sb.append(m)
            maskA_sb = cst.tile([128, 9], bf16, name="maskA_sb", tag="mA")
            nc.sync.dma_start(maskA_sb[:], maskA_d[:])
            eps_sb = cst.tile([128, 1], f32, name="eps_sb", tag="eps")
            nc.vector.memset(eps_sb[:], 1e-5)

            for s in range(BS):
                # ---- load x (c-major) + spatial-sum for SE ----
                Xs = []
                for ct in range(4):
                    Xt = xp.tile([128, HW], f32, name=f"X{s}_{ct}", tag="x")
                    nc.sync.dma_start(
                        Xt[:], x2[s, 128 * ct:128 * (ct + 1), :])
                    Xs.append(Xt)
                if s == 0:
                    for dst_t, src_ap in big_loads:
                        nc.sync.dma_start(dst_t[:], src_ap)
                svs = []
                for ct in range(4):
                    sv = smp.tile([128, 1], f32, name=f"sv{s}_{ct}", tag="sm")
                    nc.vector.tensor_reduce(sv[:], Xs[ct][:], AX.X, ALU.add)
                    svs.append(sv)
                # ---- SE: y1 = relu(w1 @ s/HW + b1) ----
                p_y1 = pts.tile([32, 1], f32, name=f"py1{s}", tag="t")
                for ct in range(4):
                    nc.tensor.matmul(p_y1[:], w1_sb[ct][:], svs[ct][:],
                                     start=(ct == 0), stop=(ct == 3))
                y1 = smp.tile([32, 1], f32, name=f"y1{s}", tag="sm")
                nc.scalar.activation(y1[:], p_y1[:], FT.Relu, bias=b1_sb[:])
                # ---- SE: y2 = sigmoid(w2 @ y1 + b2) ----
                y2cols = []
                for ct in range(4):
                    p_y2 = pts.tile([128, 1], f32, name=f"py2{s}_{ct}", tag="t")
                    nc.tensor.matmul(p_y2[:], w2_sb[:, 128 * ct:128 * (ct + 1)],
                                     y1[:], start=True, stop=True)
                    y2c = smp.tile([128, 1], f32, name=f"y2c{s}_{ct}", tag="sm")
                    nc.scalar.activation(y2c[:], p_y2[:], FT.Sigmoid,
                                         bias=b2_sb[ct][:])
                    y2cols.append(y2c)
                # y2 row [1, C] then broadcast to [128, C]
                p_row = pts.tile([1, C], f32, name=f"prow{s}", tag="t")
                for ct in range(4):
                    nc.tensor.matmul(p_row[:, 128 * ct:128 * (ct + 1)],
                                     y2cols[ct][:], ident_sb[:],
                                     start=True, stop=True)
                y2row = smp.tile([1, C], f32, name=f"y2row{s}", tag="y2r", bufs=2)
                nc.scalar.copy(y2row[:], p_row[:])
                p_y2b = pts.tile([128, C], f32, name=f"py2b{s}", tag="t")
                nc.tensor.matmul(p_y2b[:], ones_sb[:], y2row[:],
                                 start=True, stop=True)
                y2b = y2p.tile([128, C], f32, name=f"y2b{s}", tag="t")
                nc.scalar.copy(y2b[:], p_y2b[:])

                # ---- gated activations, c-major: sig_c = sigmoid(x*y2) ----
                SCs = []
                for ct in range(4):
                    oc = sgc.tile([128, HW], bf16, name=f"oc{s}_{ct}", tag="oc")
                    nc.vector.tensor_scalar_mul(oc[:], Xs[ct][:],
                                                y2cols[ct][:])
                    sc = sgc.tile([128, HW], bf16, name=f"sc{s}_{ct}", tag="sc")
                    nc.scalar.activation(sc[:], oc[:], FT.Sigmoid)
                    SCs.append(sc)

                # ---- patch logits: c-major shifted products, one-hot PE
                # reduce over channels into psum [9, 1024] (the 1/C scale is
                # folded into the one-hot weights) ----
                PAD = 64
                PW = PAD + HW + 64
                p_L = pl.tile([9, HW], f32, name=f"pL{s}", tag="t")
                n_mm = {0: 9 * 4, 1: 9 * 4}
                i_mm = {0: 0, 1: 0}
                for ct in range(4):
                    Pd = {}
                    for dpos in (0, 1, 31, 32, 33):
                        P = scrp.tile([128, PW], bf16,
                                      name=f"P{s}_{ct}_{dpos}", tag="t")
                        nc.vector.memset(P[:, 0:PAD], 0.0)
                        nc.vector.memset(P[:, PAD + HW - dpos:PW], 0.0)
                        nc.vector.tensor_tensor(
                            P[:, PAD:PAD + HW - dpos],
                            SCs[ct][:, 0:HW - dpos],
                            SCs[ct][:, dpos:HW], ALU.mult)
                        Pd[dpos] = P
                    for hh in range(2):
                        for d, k in ALLD:
                            P = Pd[abs(d)]
                            off = PAD + C * hh - (abs(d) if d < 0 else 0)
                            nc.tensor.matmul(
                                p_L[0:9, C * hh:C * (hh + 1)],
                                oneh_sb[k][:], P[:, off:off + C],
                                start=(i_mm[hh] == 0),
                                stop=(i_mm[hh] == n_mm[hh] - 1))
                            i_mm[hh] += 1
                L_c = ltp.tile([9, HW], f32, name=f"Lc{s}", tag="lc", bufs=2)
                nc.scalar.copy(L_c[0:9, 0:C], p_L[0:9, 0:C])
                nc.scalar.copy(L_c[0:9, C:HW], p_L[0:9, C:HW])

                # ---- transpose logits to q-major, mask, softmax, export ----
                for t in range(8):
                    p_lt = pts.tile([128, 9], f32, name=f"plt{s}_{t}", tag="t")
                    nc.tensor.transpose(p_lt[:],
                                        L_c[0:9, 128 * t:128 * (t + 1)],
                                        ident_sb[0:9, 0:9])
                    Lm = ltp.tile([128, 9], f32, name=f"Lm{s}_{t}", tag="t")
                    nc.vector.tensor_tensor(Lm[:], p_lt[:], maskL_sb[t][:],
                                            ALU.mult)
                    nmax = smp.tile([128, 1], f32, name=f"nm{s}_{t}", tag="sm")
                    nc.vector.tensor_reduce(nmax[:], Lm[:], AX.X, ALU.max,
                                            negate=True)
                    ee = ltp.tile([128, 9], f32, name=f"e{s}_{t}", tag="e")
                    ssum = smp.tile([128, 1], f32, name=f"ssum{s}_{t}",
                                    tag="sm")
                    nc.scalar.activation(ee[:], Lm[:], FT.Exp,
                                         bias=nmax[:], accum_out=ssum[:])
                    rin = smp.tile([128, 1], f32, name=f"rin{s}_{t}", tag="sm")
                    nc.vector.reciprocal(rin[:], ssum[:])
                    atn = anp.tile([128, 9], bf16, name=f"atn{s}_{t}", tag="t")
                    nc.vector.tensor_scalar(atn[:], ee[:], rin[:], None,
                                            ALU.mult)
                    nc.vector.tensor_mul(atn[:], atn[:], maskA_sb[:])
                    nc.scalar.dma_start(
                        attn_d[s, 128 * t:128 * (t + 1), :], atn[:])

                # ---- build banded attention matrix via diagonal DMAs ----
                at_tensor = atz_d.tensor
                an_tensor = attn_d.tensor
                with nc.allow_non_contiguous_dma(reason="diagonal scatter"):
                    for d, k in ALLD:
                        cnt = HW - abs(d)
                        q0 = max(0, -d)
                        src = AP(tensor=an_tensor,
                                 offset=s * HW * 9 + q0 * 9 + k,
                                 ap=[[9, cnt], [1, 1]])
                        dst = AP(tensor=at_tensor,
                                 offset=s * HW * HW + (q0 + d) * HW + q0,
                                 ap=[[HW + 1, cnt], [1, 1]])
                        nc.scalar.dma_start(dst, src)
                # band loads
                ATs = []
                for tk in range(8):
                    c0 = 128 * max(tk - 1, 0)
                    c1 = 128 * min(tk + 2, 8)
                    wdt = c1 - c0
                    atb = atp.tile([128, 384], bf16, name=f"atb{s}_{tk}",
                                   tag="t")
                    src = AP(tensor=at_tensor,
                             offset=s * HW * HW + 128 * tk * HW + c0,
                             ap=[[HW, 128], [1, wdt]])
                    nc.scalar.dma_start(atb[:, 0:wdt], src)
                    ATs.append((atb, c0))

                # ---- transpose x, apply gate (q-major) ----
                OTs = []
                for tk in range(8):
                    p_xt = pts.tile([128, C], f32, name=f"pxt{s}_{tk}", tag="t")
                    for ct in range(4):
                        nc.tensor.transpose(
                            p_xt[:, 128 * ct:128 * (ct + 1)],
                            Xs[ct][:, 128 * tk:128 * (tk + 1)],
                            ident_sb[:])
                    OT = otp.tile([128, C], bf16, name=f"OT{s}_{tk}", tag="t")
                    nc.vector.tensor_tensor(OT[:], p_xt[:], y2b[:], ALU.mult)
                    OTs.append(OT)

                # ---- cat rhs tiles for the gaussian half ([c', q] view) ----
                CATs = []
                for tcc in range(4):
                    cat_t = catp.tile([128, HW], bf16, name=f"CAT{s}_{tcc}",
                                      tag="t")
                    CATs.append(cat_t)

                # ---- gaussian pooling on PE ----
                # gusT columns are host-permuted (evens first) so psum
                # partitions [0:64) are even p (first q-half of channel
                # p/2) and [64:128) odd p (second half).
                for tm in range(8):
                    p_g = pg.tile([128, C], f32, name=f"pgus{s}_{tm}", tag="t")
                    for tk in range(8):
                        nc.tensor.matmul(
                            p_g[:], gus_sb[tk][:, 128 * tm:128 * (tm + 1)],
                            OTs[tk][:], start=(tk == 0), stop=(tk == 7))
                    base = (tm % 2) * 64
                    dst = CATs[tm // 2]
                    nc.scalar.copy(dst[base:base + 64, 0:C], p_g[0:64, :])
                    nc.scalar.copy(dst[base:base + 64, C:2 * C],
                                   p_g[64:128, :])


                # ---- attention apply: g2 = AT.T @ OT on PE ----
                G2s = []
                for tm in range(8):
                    p_a = pg.tile([128, C], f32, name=f"pat{s}_{tm}", tag="t")
                    ks = [t for t in (tm - 1, tm, tm + 1) if 0 <= t < 8]
                    for i, tk in enumerate(ks):
                        atb, c0 = ATs[tk]
                        coff = 128 * tm - c0
                        nc.tensor.matmul(
                            p_a[:], atb[:, coff:coff + 128], OTs[tk][:],
                            start=(i == 0), stop=(i == len(ks) - 1))
                    g2b = g2p.tile([128, C], bf16, name=f"g2b{s}_{tm}", tag="t")
                    nc.scalar.copy(g2b[:], p_a[:])
                    G2s.append(g2b)

                # ---- down conv + instance norm + leaky relu ----
                # csa half uses zero-interleaved expanded weights (d2e/d2o)
                # against g2 [q, c] directly -- the channel-pair view is
                # folded into the weights.
                for tm in range(4):
                    zparts = []
                    for hh in range(2):
                        p_z = pz.tile([128, C], f32, name=f"pz{s}_{tm}_{hh}",
                                      tag="t")
                        d2_sb = d2e_sb if hh == 0 else d2o_sb
                        for tk in range(4):
                            nc.tensor.matmul(
                                p_z[:],
                                dw1_sb[tk][:, 128 * tm:128 * (tm + 1)],
                                CATs[tk][:, C * hh:C * (hh + 1)],
                                start=(tk == 0), stop=False)
                        for tk in range(8):
                            nc.tensor.matmul(
                                p_z[:],
                                d2_sb[tk][:, 128 * tm:128 * (tm + 1)],
                                G2s[tk][:],
                                start=False, stop=(tk == 7))
                        zparts.append(p_z)
                    r0 = smp.tile([128, 1], f32, name=f"r0{s}_{tm}", tag="sm")
                    r1 = smp.tile([128, 1], f32, name=f"r1{s}_{tm}", tag="sm")
                    nc.vector.tensor_reduce(r0[:], zparts[0][:], AX.X, ALU.add)
                    nc.vector.tensor_reduce(r1[:], zparts[1][:], AX.X, ALU.add)
                    msum = smp.tile([128, 1], f32, name=f"ms{s}_{tm}", tag="sm")
                    nc.vector.tensor_add(msum[:], r0[:], r1[:])
                    sq0 = smp.tile([128, 1], f32, name=f"sq0{s}_{tm}", tag="sm")
                    sq1 = smp.tile([128, 1], f32, name=f"sq1{s}_{tm}", tag="sm")
                    sqs0 = sqp.tile([128, C], f32, name=f"sqs0{s}_{tm}",
                                    tag="t")
                    sqs1 = sqp.tile([128, C], f32, name=f"sqs1{s}_{tm}",
                                    tag="t")
                    nc.scalar.activation(sqs0[:], zparts[0][:], FT.Square,
                                         accum_out=sq0[:])
                    nc.scalar.activation(sqs1[:], zparts[1][:], FT.Square,
                                         accum_out=sq1[:])
                    sqsum = smp.tile([128, 1], f32, name=f"sqm{s}_{tm}",
                                     tag="sm")
                    nc.vector.tensor_add(sqsum[:], sq0[:], sq1[:])
                    m1 = smp.tile([128, 1], f32, name=f"m1{s}_{tm}", tag="sm")
                    nc.vector.tensor_scalar_mul(m1[:], msum[:], 1.0 / HW)
                    m2 = smp.tile([128, 1], f32, name=f"m2{s}_{tm}", tag="sm")
                    nc.vector.tensor_scalar_mul(m2[:], sqsum[:], 1.0 / HW)
                    m1sq = smp.tile([128, 1], f32, name=f"m1s{s}_{tm}",
                                    tag="sm")
                    nc.vector.tensor_mul(m1sq[:], m1[:], m1[:])
                    var = smp.tile([128, 1], f32, name=f"var{s}_{tm}", tag="sm")
                    nc.vector.tensor_sub(var[:], m2[:], m1sq[:])
                    sdev = smp.tile([128, 1], f32, name=f"sd{s}_{tm}", tag="sm")
                    nc.scalar.activation(sdev[:], var[:], FT.Sqrt,
                                         bias=eps_sb[:])
                    rstd = smp.tile([128, 1], f32, name=f"rs{s}_{tm}", tag="sm")
                    nc.vector.reciprocal(rstd[:], sdev[:])
                    nmr = smp.tile([128, 1], f32, name=f"nmr{s}_{tm}", tag="sm")
                    nc.vector.tensor_scalar(nmr[:], m1[:], rstd[:], -1.0,
                                            ALU.mult, ALU.mult)
                    for hh in range(2):
                        # zn = z*rstd - mu*rstd, then leaky = max(zn, 0.2*zn)
                        zn = sqp.tile([128, C], f32, name=f"zn{s}_{tm}_{hh}",
                                      tag="zn")
                        nc.scalar.activation(zn[:], zparts[hh][:], FT.Identity,
                                             bias=nmr[:], scale=rstd[:])
                        zs = sqp.tile([128, C], f32, name=f"zs{s}_{tm}_{hh}",
                                      tag="zs")
                        nc.vector.tensor_scalar_mul(zs[:], zn[:], 0.2)
                        zfin = zop.tile([128, C], f32, name=f"zf{s}_{tm}_{hh}",
                                        tag="t")
                        nc.vector.tensor_tensor(zfin[:], zn[:], zs[:], ALU.max)
                        nc.sync.dma_start(
                            out2[s, 128 * tm:128 * (tm + 1),
                                 C * hh:C * (hh + 1)],
                            zfin[:])

    nc.compile()
    return nc


def _host_prep(x, se_w1, se_b1, se_w2, se_b2, down_w, gus):
    bf = ml_dtypes.bfloat16
    x = np.ascontiguousarray(np.asarray(x, np.float32).reshape(B, C, HW))
    gusT = np.ascontiguousarray(np.asarray(gus, np.float32).reshape(HW, HW).T)
    # permute columns within each 128-block: evens first, then odds, so the
    # gaussian matmul's psum partitions give contiguous channel-halves
    perm = np.concatenate([np.arange(0, 128, 2), np.arange(1, 128, 2)])
    gusT = gusT.reshape(HW, 8, 128)[:, :, perm].reshape(HW, HW).astype(bf)
    dw = np.asarray(down_w, np.float32)
    dw1T = np.ascontiguousarray(dw[:, :C].T).astype(bf)           # [C, C]
    dw2 = dw[:, C:]                                               # [C, C]
    d2eT = np.zeros((HW, C), np.float32)
    d2oT = np.zeros((HW, C), np.float32)
    d2eT[0::2, :] = dw2.T
    d2oT[1::2, :] = dw2.T
    d2eT = d2eT.astype(bf)
    d2oT = d2oT.astype(bf)
    w1T = np.ascontiguousarray(np.asarray(se_w1, np.float32).T) / HW
    w2T = np.ascontiguousarray(np.asarray(se_w2, np.float32).T)
    b1 = np.asarray(se_b1, np.float32).reshape(32, 1)
    b2 = np.asarray(se_b2, np.float32).reshape(C, 1)
    ident = np.eye(128, dtype=np.float32)
    ones1 = np.ones((1, 128), np.float32)
    oneh = np.zeros((9, 128, 9), np.float32)
    for k in range(9):
        oneh[k, :, k] = 1.0 / C
    oneh = oneh.astype(bf)

    # masks over the logits tiles: [8 tiles, 128 rows, 9 offsets]
    # x-edges: x==0 kills dx=-1 cols {0,3,6}; x==31 kills dx=+1 cols {2,5,8}
    r = np.arange(128)
    maskA_x = np.ones((128, 9), np.float32)
    maskA_x[r % 32 == 0, 0] = 0
    maskA_x[r % 32 == 0, 3] = 0
    maskA_x[r % 32 == 0, 6] = 0
    maskA_x[r % 32 == 31, 2] = 0
    maskA_x[r % 32 == 31, 5] = 0
    maskA_x[r % 32 == 31, 8] = 0
    maskL = np.broadcast_to(maskA_x, (8, 128, 9)).copy()
    # y-edges: first image row (tile 0, rows<32) kills dy=-1 cols {0,1,2};
    # last image row (tile 7, rows>=96) kills dy=+1 cols {6,7,8}
    maskL[0, :32, 0:3] = 0
    maskL[7, 96:, 6:9] = 0
    # attention export mask: x-edges only; y-edge entries are excluded by
    # the diagonal DMA ranges.
    maskA_t = maskA_x.astype(bf)

    atz = np.zeros((BS, HW, HW), bf)

    shared = {
        "gusT": gusT, "dw1T": dw1T, "d2eT": d2eT, "d2oT": d2oT,
        "w1T": w1T, "w2T": w2T,
        "b1": b1, "b2": b2, "ident": ident, "ones1": ones1, "oneh": oneh,
        "maskL": maskL, "maskA": maskA_t, "atz": atz,
    }
    in_maps = []
    for i in range(NCORES):
        m = dict(shared)
        m["x2"] = np.ascontiguousarray(x[BS * i:BS * (i + 1)])
        in_maps.append(m)
    return in_maps


def _ensure_ntff_hook():
    """Wire the missing antenv.axon_hooks shim so trace=True works."""
    import sys
    import types
    try:
        import antenv.axon_hooks  # noqa: F401
        return
    except ImportError:
        pass
    from trn_agent_boot.trn_boot import _ntff_profile_via_ctypes
    hook = _ntff_profile_via_ctypes("/opt/axon/libaxon_pjrt.so")
    mod = types.ModuleType("antenv.axon_hooks")
    mod.get_axon_ntff_profile_hook = lambda: hook
    mod.set_axon_ntff_profile_hook = lambda h: None
    sys.modules["antenv.axon_hooks"] = mod


def kernel(x, se_w1, se_b1, se_w2, se_b2, down_w, gus, _trace=False):
    from concourse import bass_utils

    if _trace:
        try:
            _ensure_ntff_hook()
        except Exception:
            _trace = False
    if "nc" not in _CACHE:
        _CACHE["nc"] = _build_nc()
    nc = _CACHE["nc"]
    in_maps = _host_prep(x, se_w1, se_b1, se_w2, se_b2, down_w, gus)
    res = bass_utils.run_bass_kernel_spmd(
        nc, in_maps, core_ids=list(range(NCORES)), trace=_trace)
    _CACHE["last_result"] = res
    outs = [res.results[i]["out2"] for i in range(NCORES)]
    out = np.concatenate(outs, axis=0).reshape(B, C, H, W)
    return out.astype(np.float32)


# revision 29
# speedup vs baseline: 1.3578x; 1.2913x over previous
"""Self-contained Trainium2 Bass kernel for nn_BASE_6442450944602.

SE gating -> gaussian-weighted global pooling -> 3x3 patch self-attention
-> 1x1 conv + InstanceNorm + LeakyReLU(0.2).  b=16, C=512, H=W=32.

Data-parallel over batch: 8 NeuronCores x 2 samples.  One SPMD Bass/Tile
program; per-core inputs differ only in the x shard.

Layout strategy (per sample):
  - x loaded c-major [c,q] (q=h*w flat), PE-transposed to q-major [q,c].
  - out32^T = x^T * broadcast(y2)  (SE gate), bf16.
  - gaussian pool = gusT.T @ out32T on PE (bf16).
  - logits via fused DVE tensor_tensor_reduce on sigmoid(out32T) with
    partition-shifted operands; negative offsets are shifted copies of
    positive ones (L[q,-d] = L[q-d,+d]).
  - attention apply = banded-matmul: a 9-diagonal [1024,1024] matrix is
    materialized via 9 strided-diagonal DMAs into a zeroed DRAM scratch,
    band-loaded back, and PE computes g2 = AT.T @ out32T.
  - "torch view" channel reinterpretation ([q,c] -> [c',q]) done with
    partition-strided SBUF->SBUF gather DMAs while building the 1x1-conv
    rhs (cat).
  - down conv on PE; instance-norm stats on DVE/ACT; LeakyReLU fused into
    a single scalar-engine activation with per-partition scale/bias.
"""

import numpy as np
import ml_dtypes

B, C, H, W = 16, 512, 32, 32
HW = H * W
NCORES = 8
BS = B // NCORES  # samples per core

_CACHE = {}


def _build_nc():
    import concourse.bacc as bacc
    import concourse.bass as bass
    import concourse.mybir as mybir
    import concourse.tile as tile
    from concourse.bass_types import AP

    f32 = mybir.dt.float32
    bf16 = mybir.dt.bfloat16
    FT = mybir.ActivationFunctionType
    ALU = mybir.AluOpType
    AX = mybir.AxisListType

    nc = bacc.Bacc("TRN2", target_bir_lowering=False, debug=False)

    x2 = nc.dram_tensor("x2", [BS, C, HW], f32, kind="ExternalInput").ap()
    gusT_d = nc.dram_tensor("gusT", [HW, HW], bf16, kind="ExternalInput").ap()
    dw1T_d = nc.dram_tensor("dw1T", [C, C], bf16, kind="ExternalInput").ap()
    d2eT_d = nc.dram_tensor("d2eT", [HW, C], bf16, kind="ExternalInput").ap()
    d2oT_d = nc.dram_tensor("d2oT", [HW, C], bf16, kind="ExternalInput").ap()
    w1T_d = nc.dram_tensor("w1T", [C, 32], f32, kind="ExternalInput").ap()
    b1_d = nc.dram_tensor("b1", [32, 1], f32, kind="ExternalInput").ap()
    w2T_d = nc.dram_tensor("w2T", [32, C], f32, kind="ExternalInput").ap()
    b2_d = nc.dram_tensor("b2", [C, 1], f32, kind="ExternalInput").ap()
    ident_d = nc.dram_tensor("ident", [128, 128], f32, kind="ExternalInput").ap()
    ones_d = nc.dram_tensor("ones1", [1, 128], f32, kind="ExternalInput").ap()
    oneh_d = nc.dram_tensor("oneh", [9, 128, 9], bf16, kind="ExternalInput").ap()
    maskL_d = nc.dram_tensor("maskL", [8, 128, 9], f32, kind="ExternalInput").ap()
    maskA_d = nc.dram_tensor("maskA", [128, 9], bf16, kind="ExternalInput").ap()
    atz_d = nc.dram_tensor("atz", [BS, HW, HW], bf16, kind="ExternalInput").ap()
    attn_d = nc.dram_tensor("attn_d", [BS, HW, 9], bf16, kind="Internal").ap()
    out2 = nc.dram_tensor("out2", [BS, C, HW], f32, kind="ExternalOutput").ap()

    # positive patch offsets (delta, logits column); negatives are copies
    POSD = [(0, 4), (1, 5), (31, 6), (32, 7), (33, 8)]
    ALLD = [(-33, 0), (-32, 1), (-31, 2), (-1, 3), (0, 4),
            (1, 5), (31, 6), (32, 7), (33, 8)]

    with tile.TileContext(nc) as tc:
        from contextlib import ExitStack
        with ExitStack() as ctx:
            cst = ctx.enter_context(tc.tile_pool(name="cst", bufs=1))
            xp = ctx.enter_context(tc.tile_pool(name="xp", bufs=8))
            otp = ctx.enter_context(tc.tile_pool(name="otp", bufs=16))
            sgc = ctx.enter_context(tc.tile_pool(name="sgc", bufs=8))
            g2p = ctx.enter_context(tc.tile_pool(name="g2p", bufs=16))
            catp = ctx.enter_context(tc.tile_pool(name="catp", bufs=8))
            atp = ctx.enter_context(tc.tile_pool(name="atp", bufs=16))
            ltp = ctx.enter_context(tc.tile_pool(name="ltp", bufs=16))
            anp = ctx.enter_context(tc.tile_pool(name="anp", bufs=8))
            scrp = ctx.enter_context(tc.tile_pool(name="scrp", bufs=6))
            zop = ctx.enter_context(tc.tile_pool(name="zop", bufs=4))
            sqp = ctx.enter_context(tc.tile_pool(name="sqp", bufs=2))
            y2p = ctx.enter_context(tc.tile_pool(name="y2p", bufs=2))
            smp = ctx.enter_context(tc.tile_pool(name="smp", bufs=24))
            # PSUM pools (8 banks): pts 2 (x transposes), pg 2 (gus+g2
            # chains), pz 2 (z / SE / logit transposes), pl 2 ([9,1024])
            pts = ctx.enter_context(
                tc.tile_pool(name="pts", bufs=2, space="PSUM"))
            pg = ctx.enter_context(
                tc.tile_pool(name="pg", bufs=2, space="PSUM"))
            pz = ctx.enter_context(
                tc.tile_pool(name="pz", bufs=2, space="PSUM"))
            pl = ctx.enter_context(
                tc.tile_pool(name="pl", bufs=1, space="PSUM"))

            # ---- constants ----
            big_loads = []
            gus_sb = []
            for t in range(8):
                g = cst.tile([128, HW], bf16, name=f"gus_sb{t}", tag=f"gus{t}")
                big_loads.append((g, gusT_d[128 * t:128 * (t + 1), :]))
                gus_sb.append(g)
            dw1_sb = []
            for t in range(4):
                d = cst.tile([128, C], bf16, name=f"dw1_sb{t}", tag=f"dw1{t}")
                big_loads.append((d, dw1T_d[128 * t:128 * (t + 1), :]))
                dw1_sb.append(d)
            d2e_sb, d2o_sb = [], []
            for t in range(8):
                de = cst.tile([128, C], bf16, name=f"d2e_sb{t}", tag=f"d2e{t}")
                big_loads.append((de, d2eT_d[128 * t:128 * (t + 1), :]))
                d2e_sb.append(de)
                do = cst.tile([128, C], bf16, name=f"d2o_sb{t}", tag=f"d2o{t}")
                big_loads.append((do, d2oT_d[128 * t:128 * (t + 1), :]))
                d2o_sb.append(do)
            w1_sb = []
            for t in range(4):
                wt = cst.tile([128, 32], f32, name=f"w1_sb{t}", tag=f"w1{t}")
                nc.sync.dma_start(wt[:], w1T_d[128 * t:128 * (t + 1), :])
                w1_sb.append(wt)
            w2_sb = cst.tile([32, C], f32, name="w2_sb", tag="w2")
            nc.sync.dma_start(w2_sb[:], w2T_d[:])
            b1_sb = cst.tile([32, 1], f32, name="b1_sb", tag="b1")
            nc.sync.dma_start(b1_sb[:], b1_d[:])
            b2_sb = []
            for t in range(4):
                bt = cst.tile([128, 1], f32, name=f"b2_sb{t}", tag=f"b2{t}")
                nc.sync.dma_start(bt[:], b2_d[128 * t:128 * (t + 1), :])
                b2_sb.append(bt)
            ident_sb = cst.tile([128, 128], f32, name="ident_sb", tag="id")
            nc.sync.dma_start(ident_sb[:], ident_d[:])
            ones_sb = cst.tile([1, 128], f32, name="ones_sb", tag="on")
            nc.sync.dma_start(ones_sb[:], ones_d[:])
            oneh_sb = []
            for k in range(9):
                oh = cst.tile([128, 9], bf16, name=f"oneh_sb{k}", tag=f"oh{k}")
                nc.sync.dma_start(oh[:], oneh_d[k, :, :])
                oneh_sb.append(oh)
            maskL_sb = []
            for t in range(8):
                m = cst.tile([128, 9], f32, name=f"maskL_sb{t}", tag=f"mL{t}")
                nc.sync.dma_start(m[:], maskL_d[t, :, :])
                maskL_sb.append(m)
            maskA_sb = cst.tile([128, 9], bf16, name="maskA_sb", tag="mA")
            nc.sync.dma_start(maskA_sb[:], maskA_d[:])
            eps_sb = cst.tile([128, 1], f32, name="eps_sb", tag="eps")
            nc.vector.memset(eps_sb[:], 1e-5)

            # ---- per-sample state, staged emission so the two
            # samples interleave on every engine's in-order stream ----
            st = [dict() for _ in range(BS)]
            PAD = 64
            PW = PAD + HW + 64
            at_tensor = atz_d.tensor
            an_tensor = attn_d.tensor

            def stage_load_se(s):
                v = st[s]
                Xs = []
                for ct in range(4):
                    Xt = xp.tile([128, HW], f32, name=f"X{s}_{ct}", tag="x")
                    nc.sync.dma_start(
                        Xt[:], x2[s, 128 * ct:128 * (ct + 1), :])
                    Xs.append(Xt)
                v["Xs"] = Xs
                if s == 0:
                    for dst_t, src_ap in big_loads:
                        nc.sync.dma_start(dst_t[:], src_ap)
                svs = []
                for ct in range(4):
                    sv = smp.tile([128, 1], f32, name=f"sv{s}_{ct}", tag="sm")
                    nc.vector.tensor_reduce(sv[:], Xs[ct][:], AX.X, ALU.add)
                    svs.append(sv)
                p_y1 = pts.tile([32, 1], f32, name=f"py1{s}", tag="t")
                for ct in range(4):
                    nc.tensor.matmul(p_y1[:], w1_sb[ct][:], svs[ct][:],
                                     start=(ct == 0), stop=(ct == 3))
                y1 = smp.tile([32, 1], f32, name=f"y1{s}", tag="sm")
                nc.scalar.activation(y1[:], p_y1[:], FT.Relu, bias=b1_sb[:])
                y2cols = []
                for ct in range(4):
                    p_y2 = pts.tile([128, 1], f32, name=f"py2{s}_{ct}",
                                    tag="t")
                    nc.tensor.matmul(p_y2[:],
                                     w2_sb[:, 128 * ct:128 * (ct + 1)],
                                     y1[:], start=True, stop=True)
                    y2c = smp.tile([128, 1], f32, name=f"y2c{s}_{ct}",
                                   tag="sm")
                    nc.scalar.activation(y2c[:], p_y2[:], FT.Sigmoid,
                                         bias=b2_sb[ct][:])
                    y2cols.append(y2c)
                v["y2cols"] = y2cols
                p_row = pts.tile([1, C], f32, name=f"prow{s}", tag="t")
                for ct in range(4):
                    nc.tensor.matmul(p_row[:, 128 * ct:128 * (ct + 1)],
                                     y2cols[ct][:], ident_sb[:],
                                     start=True, stop=True)
                y2row = smp.tile([1, C], f32, name=f"y2row{s}", tag="y2r",
                                 bufs=2)
                nc.scalar.copy(y2row[:], p_row[:])
                p_y2b = pts.tile([128, C], f32, name=f"py2b{s}", tag="t")
                nc.tensor.matmul(p_y2b[:], ones_sb[:], y2row[:],
                                 start=True, stop=True)
                y2b = y2p.tile([128, C], f32, name=f"y2b{s}", tag="t")
                nc.scalar.copy(y2b[:], p_y2b[:])
                v["y2b"] = y2b
                # sigmoid(x*y2) fused on ACT: scale is the per-partition gate
                SCs = []
                for ct in range(4):
                    sc = sgc.tile([128, HW], bf16, name=f"sc{s}_{ct}",
                                  tag="sc")
                    nc.scalar.activation(sc[:], Xs[ct][:], FT.Sigmoid,
                                         scale=y2cols[ct][:])
                    SCs.append(sc)
                v["SCs"] = SCs

            def stage_xpose(s):
                v = st[s]
                Xs, y2b = v["Xs"], v["y2b"]
                OTs = []
                for tk in range(8):
                    p_xt = pts.tile([128, C], f32, name=f"pxt{s}_{tk}",
                                    tag="t")
                    for ct in range(4):
                        nc.tensor.transpose(
                            p_xt[:, 128 * ct:128 * (ct + 1)],
                            Xs[ct][:, 128 * tk:128 * (tk + 1)],
                            ident_sb[:])
                    OT = otp.tile([128, C], bf16, name=f"OT{s}_{tk}", tag="t")
                    nc.vector.tensor_tensor(OT[:], p_xt[:], y2b[:], ALU.mult)
                    OTs.append(OT)
                v["OTs"] = OTs

            def stage_logits(s):
                v = st[s]
                SCs = v["SCs"]
                p_L = pl.tile([9, HW], f32, name=f"pL{s}", tag="t")
                n_mm = 9 * 4
                i_mm = {0: 0, 1: 0}
                for ct in range(4):
                    Pd = {}
                    for dpos in (0, 1, 31, 32, 33):
                        P = scrp.tile([128, PW], bf16,
                                      name=f"P{s}_{ct}_{dpos}", tag="t")
                        nc.vector.memset(P[:, 0:PAD], 0.0)
                        nc.vector.memset(P[:, PAD + HW - dpos:PW], 0.0)
                        nc.vector.tensor_tensor(
                            P[:, PAD:PAD + HW - dpos],
                            SCs[ct][:, 0:HW - dpos],
                            SCs[ct][:, dpos:HW], ALU.mult)
                        Pd[dpos] = P
                    for hh in range(2):
                        for d, k in ALLD:
                            P = Pd[abs(d)]
                            off = PAD + C * hh - (abs(d) if d < 0 else 0)
                            nc.tensor.matmul(
                                p_L[0:9, C * hh:C * (hh + 1)],
                                oneh_sb[k][:], P[:, off:off + C],
                                start=(i_mm[hh] == 0),
                                stop=(i_mm[hh] == n_mm - 1))
                            i_mm[hh] += 1
                L_c = ltp.tile([9, HW], f32, name=f"Lc{s}", tag="lc", bufs=2)
                nc.scalar.copy(L_c[0:9, 0:C], p_L[0:9, 0:C])
                nc.scalar.copy(L_c[0:9, C:HW], p_L[0:9, C:HW])
                v["L_c"] = L_c

            def stage_attn(s):
                # per-sample DMA ring: s0 -> ACT ring, s1 -> SP ring, so the
                # two diagonal-scatter chains run concurrently and neither
                # blocks the other sample's traffic
                eng = nc.scalar if s == 0 else nc.sync
                v = st[s]
                L_c = v["L_c"]
                for t in range(8):
                    p_lt = pts.tile([128, 9], f32, name=f"plt{s}_{t}",
                                    tag="t")
                    nc.tensor.transpose(p_lt[:],
                                        L_c[0:9, 128 * t:128 * (t + 1)],
                                        ident_sb[0:9, 0:9])
                    Lm = ltp.tile([128, 9], f32, name=f"Lm{s}_{t}", tag="t")
                    nc.vector.tensor_tensor(Lm[:], p_lt[:], maskL_sb[t][:],
                                            ALU.mult)
                    nmax = smp.tile([128, 1], f32, name=f"nm{s}_{t}",
                                    tag="sm")
                    nc.vector.tensor_reduce(nmax[:], Lm[:], AX.X, ALU.max,
                                            negate=True)
                    ee = ltp.tile([128, 9], f32, name=f"e{s}_{t}", tag="e")
                    ssum = smp.tile([128, 1], f32, name=f"ssum{s}_{t}",
                                    tag="sm")
                    nc.scalar.activation(ee[:], Lm[:], FT.Exp,
                                         bias=nmax[:], accum_out=ssum[:])
                    rin = smp.tile([128, 1], f32, name=f"rin{s}_{t}",
                                   tag="sm")
                    nc.vector.reciprocal(rin[:], ssum[:])
                    atn = anp.tile([128, 9], bf16, name=f"atn{s}_{t}",
                                   tag="t")
                    nc.vector.tensor_scalar(atn[:], ee[:], rin[:], None,
                                            ALU.mult)
                    nc.vector.tensor_mul(atn[:], atn[:], maskA_sb[:])
                    eng.dma_start(attn_d[s, 128 * t:128 * (t + 1), :],
                                  atn[:])
                with nc.allow_non_contiguous_dma(reason="diagonal scatter"):
                    for d, k in ALLD:
                        cnt = HW - abs(d)
                        q0 = max(0, -d)
                        dsrc = AP(tensor=an_tensor,
                                  offset=s * HW * 9 + q0 * 9 + k,
                                  ap=[[9, cnt], [1, 1]])
                        ddst = AP(tensor=at_tensor,
                                  offset=s * HW * HW + (q0 + d) * HW + q0,
                                  ap=[[HW + 1, cnt], [1, 1]])
                        eng.dma_start(ddst, dsrc)
                ATs = []
                for tk in range(8):
                    c0 = 128 * max(tk - 1, 0)
                    c1 = 128 * min(tk + 2, 8)
                    wdt = c1 - c0
                    atb = atp.tile([128, 384], bf16, name=f"atb{s}_{tk}",
                                   tag="t")
                    bsrc = AP(tensor=at_tensor,
                              offset=s * HW * HW + 128 * tk * HW + c0,
                              ap=[[HW, 128], [1, wdt]])
                    eng.dma_start(atb[:, 0:wdt], bsrc)
                    ATs.append((atb, c0))
                v["ATs"] = ATs

            def stage_gus(s):
                v = st[s]
                OTs = v["OTs"]
                CATs = []
                for tcc in range(4):
                    cat_t = catp.tile([128, HW], bf16, name=f"CAT{s}_{tcc}",
                                      tag="t")
                    CATs.append(cat_t)
                v["CATs"] = CATs
                # gusT columns host-permuted (evens first): psum partitions
                # [0:64) even p -> first q-half of channel p/2, [64:128) odd
                for tm in range(8):
                    p_g = pg.tile([128, C], f32, name=f"pgus{s}_{tm}",
                                  tag="t")
                    for tk in range(8):
                        nc.tensor.matmul(
                            p_g[:], gus_sb[tk][:, 128 * tm:128 * (tm + 1)],
                            OTs[tk][:], start=(tk == 0), stop=(tk == 7))
                    base = (tm % 2) * 64
                    dst = CATs[tm // 2]
                    nc.scalar.copy(dst[base:base + 64, 0:C], p_g[0:64, :])
                    nc.scalar.copy(dst[base:base + 64, C:2 * C],
                                   p_g[64:128, :])

            def stage_g2(s):
                v = st[s]
                OTs, ATs = v["OTs"], v["ATs"]
                G2s = []
                for tm in range(8):
                    p_a = pg.tile([128, C], f32, name=f"pat{s}_{tm}", tag="t")
                    ks = [t for t in (tm - 1, tm, tm + 1) if 0 <= t < 8]
                    for i, tk in enumerate(ks):
                        atb, c0 = ATs[tk]
                        coff = 128 * tm - c0
                        nc.tensor.matmul(
                            p_a[:], atb[:, coff:coff + 128], OTs[tk][:],
                            start=(i == 0), stop=(i == len(ks) - 1))
                    g2b = g2p.tile([128, C], bf16, name=f"g2b{s}_{tm}",
                                   tag="t")
                    nc.scalar.copy(g2b[:], p_a[:])
                    G2s.append(g2b)
                v["G2s"] = G2s

            def stage_down(s):
                v = st[s]
                CATs, G2s = v["CATs"], v["G2s"]
                for tm in range(4):
                    zparts = []
                    for hh in range(2):
                        p_z = pz.tile([128, C], f32,
                                      name=f"pz{s}_{tm}_{hh}", tag="t")
                        d2_sb = d2e_sb if hh == 0 else d2o_sb
                        for tk in range(4):
                            nc.tensor.matmul(
                                p_z[:],
                                dw1_sb[tk][:, 128 * tm:128 * (tm + 1)],
                                CATs[tk][:, C * hh:C * (hh + 1)],
                                start=(tk == 0), stop=False)
                        for tk in range(8):
                            nc.tensor.matmul(
                                p_z[:],
                                d2_sb[tk][:, 128 * tm:128 * (tm + 1)],
                                G2s[tk][:],
                                start=False, stop=(tk == 7))
                        zparts.append(p_z)
                    r0 = smp.tile([128, 1], f32, name=f"r0{s}_{tm}", tag="sm")
                    r1 = smp.tile([128, 1], f32, name=f"r1{s}_{tm}", tag="sm")
                    nc.vector.tensor_reduce(r0[:], zparts[0][:], AX.X,
                                            ALU.add)
                    nc.vector.tensor_reduce(r1[:], zparts[1][:], AX.X,
                                            ALU.add)
                    msum = smp.tile([128, 1], f32, name=f"ms{s}_{tm}",
                                    tag="sm")
                    nc.vector.tensor_add(msum[:], r0[:], r1[:])
                    sq0 = smp.tile([128, 1], f32, name=f"sq0{s}_{tm}",
                                   tag="sm")
                    sq1 = smp.tile([128, 1], f32, name=f"sq1{s}_{tm}",
                                   tag="sm")
                    sqs0 = sqp.tile([128, C], f32, name=f"sqs0{s}_{tm}",
                                    tag="t")
                    sqs1 = sqp.tile([128, C], f32, name=f"sqs1{s}_{tm}",
                                    tag="t")
                    nc.scalar.activation(sqs0[:], zparts[0][:], FT.Square,
                                         accum_out=sq0[:])
                    nc.scalar.activation(sqs1[:], zparts[1][:], FT.Square,
                                         accum_out=sq1[:])
                    sqsum = smp.tile([128, 1], f32, name=f"sqm{s}_{tm}",
                                     tag="sm")
                    nc.vector.tensor_add(sqsum[:], sq0[:], sq1[:])
                    m1 = smp.tile([128, 1], f32, name=f"m1{s}_{tm}", tag="sm")
                    nc.vector.tensor_scalar_mul(m1[:], msum[:], 1.0 / HW)
                    m2 = smp.tile([128, 1], f32, name=f"m2{s}_{tm}", tag="sm")
                    nc.vector.tensor_scalar_mul(m2[:], sqsum[:], 1.0 / HW)
                    m1sq = smp.tile([128, 1], f32, name=f"m1s{s}_{tm}",
                                    tag="sm")
                    nc.vector.tensor_mul(m1sq[:], m1[:], m1[:])
                    var = smp.tile([128, 1], f32, name=f"var{s}_{tm}",
                                   tag="sm")
                    nc.vector.tensor_sub(var[:], m2[:], m1sq[:])
                    sdev = smp.tile([128, 1], f32, name=f"sd{s}_{tm}",
                                    tag="sm")
                    nc.scalar.activation(sdev[:], var[:], FT.Sqrt,
                                         bias=eps_sb[:])
                    rstd = smp.tile([128, 1], f32, name=f"rs{s}_{tm}",
                                    tag="sm")
                    nc.vector.reciprocal(rstd[:], sdev[:])
                    nmr = smp.tile([128, 1], f32, name=f"nmr{s}_{tm}",
                                   tag="sm")
                    nc.vector.tensor_scalar(nmr[:], m1[:], rstd[:], -1.0,
                                            ALU.mult, ALU.mult)
                    for hh in range(2):
                        zn = sqp.tile([128, C], f32,
                                      name=f"zn{s}_{tm}_{hh}", tag="zn")
                        nc.scalar.activation(zn[:], zparts[hh][:],
                                             FT.Identity,
                                             bias=nmr[:], scale=rstd[:])
                        zs = sqp.tile([128, C], f32,
                                      name=f"zs{s}_{tm}_{hh}", tag="zs")
                        nc.vector.tensor_scalar_mul(zs[:], zn[:], 0.2)
                        zfin = zop.tile([128, C], f32,
                                        name=f"zf{s}_{tm}_{hh}", tag="t")
                        nc.vector.tensor_tensor(zfin[:], zn[:], zs[:],
                                                ALU.max)
                        nc.sync.dma_start(
                            out2[s, 128 * tm:128 * (tm + 1),
                                 C * hh:C * (hh + 1)],
                            zfin[:])

            for stage in (stage_load_se, stage_xpose, stage_logits,
                          stage_attn, stage_gus, stage_g2, stage_down):
                for s in range(BS):
                    stage(s)

    nc.compile()
    return nc


def _host_prep(x, se_w1, se_b1, se_w2, se_b2, down_w, gus):
    bf = ml_dtypes.bfloat16
    x = np.ascontiguousarray(np.asarray(x, np.float32).reshape(B, C, HW))
    gusT = np.ascontiguousarray(np.asarray(gus, np.float32).reshape(HW, HW).T)
    # permute columns within each 128-block: evens first, then odds, so the
    # gaussian matmul's psum partitions give contiguous channel-halves
    perm = np.concatenate([np.arange(0, 128, 2), np.arange(1, 128, 2)])
    gusT = gusT.reshape(HW, 8, 128)[:, :, perm].reshape(HW, HW).astype(bf)
    dw = np.asarray(down_w, np.float32)
    dw1T = np.ascontiguousarray(dw[:, :C].T).astype(bf)           # [C, C]
    dw2 = dw[:, C:]                                               # [C, C]
    d2eT = np.zeros((HW, C), np.float32)
    d2oT = np.zeros((HW, C), np.float32)
    d2eT[0::2, :] = dw2.T
    d2oT[1::2, :] = dw2.T
    d2eT = d2eT.astype(bf)
    d2oT = d2oT.astype(bf)
    w1T = np.ascontiguousarray(np.asarray(se_w1, np.float32).T) / HW
    w2T = np.ascontiguousarray(np.asarray(se_w2, np.float32).T)
    b1 = np.asarray(se_b1, np.float32).reshape(32, 1)
    b2 = np.asarray(se_b2, np.float32).reshape(C, 1)
    ident = np.eye(128, dtype=np.float32)
    ones1 = np.ones((1, 128), np.float32)
    oneh = np.zeros((9, 128, 9), np.float32)
    for k in range(9):
        oneh[k, :, k] = 1.0 / C
    oneh = oneh.astype(bf)

    # masks over the logits tiles: [8 tiles, 128 rows, 9 offsets]
    # x-edges: x==0 kills dx=-1 cols {0,3,6}; x==31 kills dx=+1 cols {2,5,8}
    r = np.arange(128)
    maskA_x = np.ones((128, 9), np.float32)
    maskA_x[r % 32 == 0, 0] = 0
    maskA_x[r % 32 == 0, 3] = 0
    maskA_x[r % 32 == 0, 6] = 0
    maskA_x[r % 32 == 31, 2] = 0
    maskA_x[r % 32 == 31, 5] = 0
    maskA_x[r % 32 == 31, 8] = 0
    maskL = np.broadcast_to(maskA_x, (8, 128, 9)).copy()
    # y-edges: first image row (tile 0, rows<32) kills dy=-1 cols {0,1,2};
    # last image row (tile 7, rows>=96) kills dy=+1 cols {6,7,8}
    maskL[0, :32, 0:3] = 0
    maskL[7, 96:, 6:9] = 0
    # attention export mask: x-edges only; y-edge entries are excluded by
    # the diagonal DMA ranges.
    maskA_t = maskA_x.astype(bf)

    atz = np.zeros((BS, HW, HW), bf)

    shared = {
        "gusT": gusT, "dw1T": dw1T, "d2eT": d2eT, "d2oT": d2oT,
        "w1T": w1T, "w2T": w2T,
        "b1": b1, "b2": b2, "ident": ident, "ones1": ones1, "oneh": oneh,
        "maskL": maskL, "maskA": maskA_t, "atz": atz,
    }
    in_maps = []
    for i in range(NCORES):
        m = dict(shared)
        m["x2"] = np.ascontiguousarray(x[BS * i:BS * (i + 1)])
        in_maps.append(m)
    return in_maps


def _ensure_ntff_hook():
    """Wire the missing antenv.axon_hooks shim so trace=True works."""
    import sys
    import types
    try:
        import antenv.axon_hooks  # noqa: F401
        return
    except ImportError:
        pass
    from trn_agent_boot.trn_boot import _ntff_profile_via_ctypes
    hook = _ntff_profile_via_ctypes("/opt/axon/libaxon_pjrt.so")
    mod = types.ModuleType("antenv.axon_hooks")
    mod.get_axon_ntff_profile_hook = lambda: hook
    mod.set_axon_ntff_profile_hook = lambda h: None
    sys.modules["antenv.axon_hooks"] = mod


def kernel(x, se_w1, se_b1, se_w2, se_b2, down_w, gus, _trace=False):
    from concourse import bass_utils

    if _trace:
        try:
            _ensure_ntff_hook()
        except Exception:
            _trace = False
    if "nc" not in _CACHE:
        _CACHE["nc"] = _build_nc()
    nc = _CACHE["nc"]
    in_maps = _host_prep(x, se_w1, se_b1, se_w2, se_b2, down_w, gus)
    res = bass_utils.run_bass_kernel_spmd(
        nc, in_maps, core_ids=list(range(NCORES)), trace=_trace)
    _CACHE["last_result"] = res
    outs = [res.results[i]["out2"] for i in range(NCORES)]
    out = np.concatenate(outs, axis=0).reshape(B, C, H, W)
    return out.astype(np.float32)


# revision 35
# speedup vs baseline: 1.4495x; 1.0675x over previous
"""Self-contained Trainium2 Bass kernel for nn_BASE_6442450944602.

SE gating -> gaussian-weighted global pooling -> 3x3 patch self-attention
-> 1x1 conv + InstanceNorm + LeakyReLU(0.2).  b=16, C=512, H=W=32.

Data-parallel over batch: 8 NeuronCores x 2 samples.  One SPMD Bass/Tile
program; per-core inputs differ only in the x shard.

Layout strategy (per sample):
  - x loaded c-major [c,q] (q=h*w flat), PE-transposed to q-major [q,c].
  - out32^T = x^T * broadcast(y2)  (SE gate), bf16.
  - gaussian pool = gusT.T @ out32T on PE (bf16).
  - logits via fused DVE tensor_tensor_reduce on sigmoid(out32T) with
    partition-shifted operands; negative offsets are shifted copies of
    positive ones (L[q,-d] = L[q-d,+d]).
  - attention apply = banded-matmul: a 9-diagonal [1024,1024] matrix is
    materialized via 9 strided-diagonal DMAs into a zeroed DRAM scratch,
    band-loaded back, and PE computes g2 = AT.T @ out32T.
  - "torch view" channel reinterpretation ([q,c] -> [c',q]) done with
    partition-strided SBUF->SBUF gather DMAs while building the 1x1-conv
    rhs (cat).
  - down conv on PE; instance-norm stats on DVE/ACT; LeakyReLU fused into
    a single scalar-engine activation with per-partition scale/bias.
"""

import numpy as np
import ml_dtypes

B, C, H, W = 16, 512, 32, 32
HW = H * W
NCORES = 8
BS = B // NCORES  # samples per core

_CACHE = {}


def _build_nc():
    import concourse.bacc as bacc
    import concourse.bass as bass
    import concourse.mybir as mybir
    import concourse.tile as tile
    from concourse.bass_types import AP

    f32 = mybir.dt.float32
    bf16 = mybir.dt.bfloat16
    FT = mybir.ActivationFunctionType
    ALU = mybir.AluOpType
    AX = mybir.AxisListType

    nc = bacc.Bacc("TRN2", target_bir_lowering=False, debug=False)

    x2 = nc.dram_tensor("x2", [BS, C, HW], bf16, kind="ExternalInput").ap()
    gusT_d = nc.dram_tensor("gusT", [HW, HW], bf16, kind="ExternalInput").ap()
    dw1T_d = nc.dram_tensor("dw1T", [C, C], bf16, kind="ExternalInput").ap()
    d2eT_d = nc.dram_tensor("d2eT", [HW, C], bf16, kind="ExternalInput").ap()
    d2oT_d = nc.dram_tensor("d2oT", [HW, C], bf16, kind="ExternalInput").ap()
    w1T_d = nc.dram_tensor("w1T", [C, 32], f32, kind="ExternalInput").ap()
    b1_d = nc.dram_tensor("b1", [32, 1], f32, kind="ExternalInput").ap()
    w2T_d = nc.dram_tensor("w2T", [32, C], f32, kind="ExternalInput").ap()
    b2_d = nc.dram_tensor("b2", [C, 1], f32, kind="ExternalInput").ap()
    ident_d = nc.dram_tensor("ident", [128, 128], f32, kind="ExternalInput").ap()
    identb_d = nc.dram_tensor("identb", [128, 128], bf16,
                              kind="ExternalInput").ap()
    ones_d = nc.dram_tensor("ones1", [1, 128], f32, kind="ExternalInput").ap()
    oneh_d = nc.dram_tensor("oneh", [9, 128, 9], bf16, kind="ExternalInput").ap()
    maskL_d = nc.dram_tensor("maskL", [8, 128, 9], f32, kind="ExternalInput").ap()
    maskA_d = nc.dram_tensor("maskA", [128, 9], bf16, kind="ExternalInput").ap()
    atz_d = nc.dram_tensor("atz", [BS, HW, HW], bf16, kind="ExternalInput").ap()
    attn_d = nc.dram_tensor("attn_d", [BS, HW, 9], bf16, kind="Internal").ap()
    out2 = nc.dram_tensor("out2", [BS, C, HW], f32, kind="ExternalOutput").ap()

    # positive patch offsets (delta, logits column); negatives are copies
    POSD = [(0, 4), (1, 5), (31, 6), (32, 7), (33, 8)]
    ALLD = [(-33, 0), (-32, 1), (-31, 2), (-1, 3), (0, 4),
            (1, 5), (31, 6), (32, 7), (33, 8)]

    with tile.TileContext(nc) as tc:
        from contextlib import ExitStack
        with ExitStack() as ctx:
            cst = ctx.enter_context(tc.tile_pool(name="cst", bufs=1))
            xp = ctx.enter_context(tc.tile_pool(name="xp", bufs=8))
            otp = ctx.enter_context(tc.tile_pool(name="otp", bufs=16))
            sgc = ctx.enter_context(tc.tile_pool(name="sgc", bufs=8))
            g2p = ctx.enter_context(tc.tile_pool(name="g2p", bufs=16))
            catp = ctx.enter_context(tc.tile_pool(name="catp", bufs=8))
            atp = ctx.enter_context(tc.tile_pool(name="atp", bufs=16))
            ltp = ctx.enter_context(tc.tile_pool(name="ltp", bufs=16))
            anp = ctx.enter_context(tc.tile_pool(name="anp", bufs=8))
            scrp = ctx.enter_context(tc.tile_pool(name="scrp", bufs=6))
            zop = ctx.enter_context(tc.tile_pool(name="zop", bufs=4))
            sqp = ctx.enter_context(tc.tile_pool(name="sqp", bufs=2))
            smp = ctx.enter_context(tc.tile_pool(name="smp", bufs=16))
            # PSUM pools (8 banks): pts 2 (x transposes), pg 2 (gus+g2
            # chains), pz 2 (z / SE / logit transposes), pl 2 ([9,1024])
            pts = ctx.enter_context(
                tc.tile_pool(name="pts", bufs=2, space="PSUM"))
            pg = ctx.enter_context(
                tc.tile_pool(name="pg", bufs=2, space="PSUM"))
            pz = ctx.enter_context(
                tc.tile_pool(name="pz", bufs=2, space="PSUM"))
            pl = ctx.enter_context(
                tc.tile_pool(name="pl", bufs=1, space="PSUM"))

            # ---- constants ----
            big_loads = []
            gus_sb = []
            for t in range(8):
                g = cst.tile([128, HW], bf16, name=f"gus_sb{t}", tag=f"gus{t}")
                big_loads.append((g, gusT_d[128 * t:128 * (t + 1), :]))
                gus_sb.append(g)
            dw1_sb = []
            for t in range(4):
                d = cst.tile([128, C], bf16, name=f"dw1_sb{t}", tag=f"dw1{t}")
                big_loads.append((d, dw1T_d[128 * t:128 * (t + 1), :]))
                dw1_sb.append(d)
            d2e_sb, d2o_sb = [], []
            for t in range(8):
                de = cst.tile([128, C], bf16, name=f"d2e_sb{t}", tag=f"d2e{t}")
                big_loads.append((de, d2eT_d[128 * t:128 * (t + 1), :]))
                d2e_sb.append(de)
                do = cst.tile([128, C], bf16, name=f"d2o_sb{t}", tag=f"d2o{t}")
                big_loads.append((do, d2oT_d[128 * t:128 * (t + 1), :]))
                d2o_sb.append(do)
            w1_sb = []
            for t in range(4):
                wt = cst.tile([128, 32], f32, name=f"w1_sb{t}", tag=f"w1{t}")
                nc.sync.dma_start(wt[:], w1T_d[128 * t:128 * (t + 1), :])
                w1_sb.append(wt)
            w2_sb = cst.tile([32, C], f32, name="w2_sb", tag="w2")
            nc.sync.dma_start(w2_sb[:], w2T_d[:])
            b1_sb = cst.tile([32, 1], f32, name="b1_sb", tag="b1")
            nc.sync.dma_start(b1_sb[:], b1_d[:])
            b2_sb = []
            for t in range(4):
                bt = cst.tile([128, 1], f32, name=f"b2_sb{t}", tag=f"b2{t}")
                nc.sync.dma_start(bt[:], b2_d[128 * t:128 * (t + 1), :])
                b2_sb.append(bt)
            ident_sb = cst.tile([128, 128], f32, name="ident_sb", tag="id")
            nc.sync.dma_start(ident_sb[:], ident_d[:])
            identb_sb = cst.tile([128, 128], bf16, name="identb_sb",
                                 tag="idb")
            nc.sync.dma_start(identb_sb[:], identb_d[:])
            ones_sb = cst.tile([1, 128], f32, name="ones_sb", tag="on")
            nc.sync.dma_start(ones_sb[:], ones_d[:])
            oneh_sb = []
            for k in range(9):
                oh = cst.tile([128, 9], bf16, name=f"oneh_sb{k}", tag=f"oh{k}")
                nc.sync.dma_start(oh[:], oneh_d[k, :, :])
                oneh_sb.append(oh)
            maskL_sb = []
            for t in range(8):
                m = cst.tile([128, 9], f32, name=f"maskL_sb{t}", tag=f"mL{t}")
                nc.sync.dma_start(m[:], maskL_d[t, :, :])
                maskL_sb.append(m)
            maskA_sb = cst.tile([128, 9], bf16, name="maskA_sb", tag="mA")
            nc.sync.dma_start(maskA_sb[:], maskA_d[:])
            eps_sb = cst.tile([128, 1], f32, name="eps_sb", tag="eps")
            nc.vector.memset(eps_sb[:], 1e-5)

            # ---- per-sample state, staged emission so the two
            # samples interleave on every engine's in-order stream ----
            st = [dict() for _ in range(BS)]
            PAD = 64
            PW = PAD + HW + 64
            at_tensor = atz_d.tensor
            an_tensor = attn_d.tensor

            def stage_load(s):
                v = st[s]
                Xs = []
                for ct in range(4):
                    Xt = xp.tile([128, HW], bf16, name=f"X{s}_{ct}", tag="x")
                    nc.sync.dma_start(
                        Xt[:], x2[s, 128 * ct:128 * (ct + 1), :])
                    Xs.append(Xt)
                v["Xs"] = Xs
                if s == BS - 1:
                    for dst_t, src_ap in big_loads:
                        nc.sync.dma_start(dst_t[:], src_ap)

            def stage_se(s):
                v = st[s]
                Xs = v["Xs"]
                svs = []
                for ct in range(4):
                    sv = smp.tile([128, 1], f32, name=f"sv{s}_{ct}", tag="sm")
                    nc.vector.tensor_reduce(sv[:], Xs[ct][:], AX.X, ALU.add)
                    svs.append(sv)
                p_y1 = pts.tile([32, 1], f32, name=f"py1{s}", tag="t")
                for ct in range(4):
                    nc.tensor.matmul(p_y1[:], w1_sb[ct][:], svs[ct][:],
                                     start=(ct == 0), stop=(ct == 3))
                y1 = smp.tile([32, 1], f32, name=f"y1{s}", tag="sm")
                nc.scalar.activation(y1[:], p_y1[:], FT.Relu, bias=b1_sb[:])
                y2cols = []
                for ct in range(4):
                    p_y2 = pts.tile([128, 1], f32, name=f"py2{s}_{ct}",
                                    tag="t")
                    nc.tensor.matmul(p_y2[:],
                                     w2_sb[:, 128 * ct:128 * (ct + 1)],
                                     y1[:], start=True, stop=True)
                    y2c = smp.tile([128, 1], f32, name=f"y2c{s}_{ct}",
                                   tag="sm")
                    nc.scalar.activation(y2c[:], p_y2[:], FT.Sigmoid,
                                         bias=b2_sb[ct][:])
                    y2cols.append(y2c)
                v["y2cols"] = y2cols
                # gated activations, bf16 c-major (transpose source), and
                # sigmoid(x*y2) fused on ACT with the per-partition gate
                OCs, SCs = [], []
                for ct in range(4):
                    oc = sgc.tile([128, HW], bf16, name=f"oc{s}_{ct}",
                                  tag="oc", bufs=6)
                    nc.vector.tensor_scalar_mul(oc[:], Xs[ct][:],
                                                y2cols[ct][:])
                    OCs.append(oc)
                    sc = sgc.tile([128, HW], bf16, name=f"sc{s}_{ct}",
                                  tag="sc")
                    nc.scalar.activation(sc[:], Xs[ct][:], FT.Sigmoid,
                                         scale=y2cols[ct][:])
                    SCs.append(sc)
                v["OCs"] = OCs
                v["SCs"] = SCs

            def stage_xpose(s):
                v = st[s]
                OCs = v["OCs"]
                OTs = []
                for tk in range(8):
                    p_xt = pts.tile([128, C], bf16, name=f"pxt{s}_{tk}",
                                    tag="t")
                    for ct in range(4):
                        nc.tensor.transpose(
                            p_xt[:, 128 * ct:128 * (ct + 1)],
                            OCs[ct][:, 128 * tk:128 * (tk + 1)],
                            identb_sb[:])
                    OT = otp.tile([128, C], bf16, name=f"OT{s}_{tk}", tag="t")
                    nc.scalar.copy(OT[:], p_xt[:])
                    OTs.append(OT)
                v["OTs"] = OTs

            def stage_logits(s):
                v = st[s]
                SCs = v["SCs"]
                p_L = pl.tile([9, HW], f32, name=f"pL{s}", tag="t")
                PSd = {}
                for dpos in (0, 1, 31, 32, 33):
                    Ps = []
                    for ct in range(4):
                        P = scrp.tile([128, PW], bf16,
                                      name=f"P{s}_{ct}_{dpos}", tag="t",
                                      bufs=5)
                        nc.vector.memset(P[:, 0:PAD], 0.0)
                        nc.vector.memset(P[:, PAD + HW - dpos:PW], 0.0)
                        nc.vector.tensor_tensor(
                            P[:, PAD:PAD + HW - dpos],
                            SCs[ct][:, 0:HW - dpos],
                            SCs[ct][:, dpos:HW], ALU.mult)
                        Ps.append(P)
                    # channel-tile pre-add so the PE reduce is 1 mm per
                    # (offset, half) instead of 4
                    pa01 = scrp.tile([128, PW], bf16,
                                     name=f"pa01{s}_{dpos}", tag="a", bufs=3)
                    nc.vector.tensor_add(pa01[:], Ps[0][:], Ps[1][:])
                    pa23 = scrp.tile([128, PW], bf16,
                                     name=f"pa23{s}_{dpos}", tag="a", bufs=3)
                    nc.vector.tensor_add(pa23[:], Ps[2][:], Ps[3][:])
                    PS = scrp.tile([128, PW], bf16,
                                   name=f"PS{s}_{dpos}", tag="ps")
                    nc.vector.tensor_add(PS[:], pa01[:], pa23[:])
                    PSd[dpos] = PS
                i_mm = {0: 0, 1: 0}
                for hh in range(2):
                    for d, k in ALLD:
                        PS = PSd[abs(d)]
                        off = PAD + C * hh - (abs(d) if d < 0 else 0)
                        nc.tensor.matmul(
                            p_L[0:9, C * hh:C * (hh + 1)],
                            oneh_sb[k][:], PS[:, off:off + C],
                            start=(i_mm[hh] == 0),
                            stop=(i_mm[hh] == 8))
                        i_mm[hh] += 1
                L_c = ltp.tile([9, HW], f32, name=f"Lc{s}", tag="lc", bufs=2)
                nc.scalar.copy(L_c[0:9, 0:C], p_L[0:9, 0:C])
                nc.scalar.copy(L_c[0:9, C:HW], p_L[0:9, C:HW])
                v["L_c"] = L_c

            def stage_attn(s):
                # per-sample DMA ring: s0 -> SP ring, s1 -> ACT ring, so the
                # two diagonal-scatter chains run concurrently and neither
                # blocks the other sample's traffic
                eng = nc.sync if s == 0 else nc.scalar
                v = st[s]
                L_c = v["L_c"]
                for t in range(8):
                    p_lt = pts.tile([128, 9], f32, name=f"plt{s}_{t}",
                                    tag="t")
                    nc.tensor.transpose(p_lt[:],
                                        L_c[0:9, 128 * t:128 * (t + 1)],
                                        ident_sb[0:9, 0:9])
                    Lm = ltp.tile([128, 9], f32, name=f"Lm{s}_{t}", tag="t")
                    nc.vector.tensor_tensor(Lm[:], p_lt[:], maskL_sb[t][:],
                                            ALU.mult)
                    nmax = smp.tile([128, 1], f32, name=f"nm{s}_{t}",
                                    tag="sm")
                    nc.vector.tensor_reduce(nmax[:], Lm[:], AX.X, ALU.max,
                                            negate=True)
                    ee = ltp.tile([128, 9], f32, name=f"e{s}_{t}", tag="e", bufs=8)
                    ssum = smp.tile([128, 1], f32, name=f"ssum{s}_{t}",
                                    tag="sm")
                    nc.scalar.activation(ee[:], Lm[:], FT.Exp,
                                         bias=nmax[:], accum_out=ssum[:])
                    rin = smp.tile([128, 1], f32, name=f"rin{s}_{t}",
                                   tag="sm")
                    nc.vector.reciprocal(rin[:], ssum[:])
                    atn = anp.tile([128, 9], bf16, name=f"atn{s}_{t}",
                                   tag="t")
                    nc.vector.tensor_scalar(atn[:], ee[:], rin[:], None,
                                            ALU.mult)
                    nc.vector.tensor_mul(atn[:], atn[:], maskA_sb[:])
                    eng.dma_start(attn_d[s, 128 * t:128 * (t + 1), :],
                                  atn[:])
                with nc.allow_non_contiguous_dma(reason="diagonal scatter"):
                    for d, k in ALLD:
                        cnt = HW - abs(d)
                        q0 = max(0, -d)
                        dsrc = AP(tensor=an_tensor,
                                  offset=s * HW * 9 + q0 * 9 + k,
                                  ap=[[9, cnt], [1, 1]])
                        ddst = AP(tensor=at_tensor,
                                  offset=s * HW * HW + (q0 + d) * HW + q0,
                                  ap=[[HW + 1, cnt], [1, 1]])
                        eng.dma_start(ddst, dsrc)
                ATs = []
                for tk in range(8):
                    c0 = 128 * max(tk - 1, 0)
                    c1 = 128 * min(tk + 2, 8)
                    wdt = c1 - c0
                    atb = atp.tile([128, 384], bf16, name=f"atb{s}_{tk}",
                                   tag="t")
                    bsrc = AP(tensor=at_tensor,
                              offset=s * HW * HW + 128 * tk * HW + c0,
                              ap=[[HW, 128], [1, wdt]])
                    eng.dma_start(atb[:, 0:wdt], bsrc)
                    ATs.append((atb, c0))
                v["ATs"] = ATs

            def stage_gus(s):
                v = st[s]
                OTs = v["OTs"]
                CATs = []
                for tcc in range(4):
                    cat_t = catp.tile([128, HW], bf16, name=f"CAT{s}_{tcc}",
                                      tag="t")
                    CATs.append(cat_t)
                v["CATs"] = CATs
                # gusT columns host-permuted (evens first): psum partitions
                # [0:64) even p -> first q-half of channel p/2, [64:128) odd
                for tm in range(8):
                    p_g = pg.tile([128, C], f32, name=f"pgus{s}_{tm}",
                                  tag="t")
                    for tk in range(8):
                        nc.tensor.matmul(
                            p_g[:], gus_sb[tk][:, 128 * tm:128 * (tm + 1)],
                            OTs[tk][:], start=(tk == 0), stop=(tk == 7))
                    base = (tm % 2) * 64
                    dst = CATs[tm // 2]
                    nc.vector.tensor_copy(dst[base:base + 64, 0:C],
                                          p_g[0:64, :])
                    nc.vector.tensor_copy(dst[base:base + 64, C:2 * C],
                                          p_g[64:128, :])

            def stage_g2(s):
                v = st[s]
                OTs, ATs = v["OTs"], v["ATs"]
                G2s = []
                for tm in range(8):
                    p_a = pg.tile([128, C], f32, name=f"pat{s}_{tm}", tag="t")
                    ks = [t for t in (tm - 1, tm, tm + 1) if 0 <= t < 8]
                    for i, tk in enumerate(ks):
                        atb, c0 = ATs[tk]
                        coff = 128 * tm - c0
                        nc.tensor.matmul(
                            p_a[:], atb[:, coff:coff + 128], OTs[tk][:],
                            start=(i == 0), stop=(i == len(ks) - 1))
                    g2b = g2p.tile([128, C], bf16, name=f"g2b{s}_{tm}",
                                   tag="t")
                    nc.vector.tensor_copy(g2b[:], p_a[:])
                    G2s.append(g2b)
                v["G2s"] = G2s

            def stage_down(s):
                v = st[s]
                CATs, G2s = v["CATs"], v["G2s"]
                for tm in range(4):
                    zparts = []
                    for hh in range(2):
                        p_z = pz.tile([128, C], f32,
                                      name=f"pz{s}_{tm}_{hh}", tag="t")
                        d2_sb = d2e_sb if hh == 0 else d2o_sb
                        for tk in range(4):
                            nc.tensor.matmul(
                                p_z[:],
                                dw1_sb[tk][:, 128 * tm:128 * (tm + 1)],
                                CATs[tk][:, C * hh:C * (hh + 1)],
                                start=(tk == 0), stop=False)
                        for tk in range(8):
                            nc.tensor.matmul(
                                p_z[:],
                                d2_sb[tk][:, 128 * tm:128 * (tm + 1)],
                                G2s[tk][:],
                                start=False, stop=(tk == 7))
                        zparts.append(p_z)
                    r0 = smp.tile([128, 1], f32, name=f"r0{s}_{tm}", tag="sm")
                    r1 = smp.tile([128, 1], f32, name=f"r1{s}_{tm}", tag="sm")
                    nc.vector.tensor_reduce(r0[:], zparts[0][:], AX.X,
                                            ALU.add)
                    nc.vector.tensor_reduce(r1[:], zparts[1][:], AX.X,
                                            ALU.add)
                    msum = smp.tile([128, 1], f32, name=f"ms{s}_{tm}",
                                    tag="sm")
                    nc.vector.tensor_add(msum[:], r0[:], r1[:])
                    sq0 = smp.tile([128, 1], f32, name=f"sq0{s}_{tm}",
                                   tag="sm")
                    sq1 = smp.tile([128, 1], f32, name=f"sq1{s}_{tm}",
                                   tag="sm")
                    sqs0 = zop.tile([128, C], f32, name=f"sqs0{s}_{tm}",
                                    tag="t")
                    sqs1 = zop.tile([128, C], f32, name=f"sqs1{s}_{tm}",
                                    tag="t")
                    nc.scalar.activation(sqs0[:], zparts[0][:], FT.Square,
                                         accum_out=sq0[:])
                    nc.scalar.activation(sqs1[:], zparts[1][:], FT.Square,
                                         accum_out=sq1[:])
                    sqsum = smp.tile([128, 1], f32, name=f"sqm{s}_{tm}",
                                     tag="sm")
                    nc.vector.tensor_add(sqsum[:], sq0[:], sq1[:])
                    m1 = smp.tile([128, 1], f32, name=f"m1{s}_{tm}", tag="sm")
                    nc.vector.tensor_scalar_mul(m1[:], msum[:], 1.0 / HW)
                    m2 = smp.tile([128, 1], f32, name=f"m2{s}_{tm}", tag="sm")
                    nc.vector.tensor_scalar_mul(m2[:], sqsum[:], 1.0 / HW)
                    m1sq = smp.tile([128, 1], f32, name=f"m1s{s}_{tm}",
                                    tag="sm")
                    nc.vector.tensor_mul(m1sq[:], m1[:], m1[:])
                    var = smp.tile([128, 1], f32, name=f"var{s}_{tm}",
                                   tag="sm")
                    nc.vector.tensor_sub(var[:], m2[:], m1sq[:])
                    sdev = smp.tile([128, 1], f32, name=f"sd{s}_{tm}",
                                    tag="sm")
                    nc.scalar.activation(sdev[:], var[:], FT.Sqrt,
                                         bias=eps_sb[:])
                    rstd = smp.tile([128, 1], f32, name=f"rs{s}_{tm}",
                                    tag="sm")
                    nc.vector.reciprocal(rstd[:], sdev[:])
                    nmr = smp.tile([128, 1], f32, name=f"nmr{s}_{tm}",
                                   tag="sm")
                    nc.vector.tensor_scalar(nmr[:], m1[:], rstd[:], -1.0,
                                            ALU.mult, ALU.mult)
                    for hh in range(2):
                        zn = sqp.tile([128, C], f32,
                                      name=f"zn{s}_{tm}_{hh}", tag="zn")
                        nc.scalar.activation(zn[:], zparts[hh][:],
                                             FT.Identity,
                                             bias=nmr[:], scale=rstd[:])
                        zfin = zop.tile([128, C], f32,
                                        name=f"zf{s}_{tm}_{hh}", tag="t")
                        nc.vector.scalar_tensor_tensor(
                            zfin[:], zn[:], 0.2, zn[:], ALU.mult, ALU.max)
                        nc.sync.dma_start(
                            out2[s, 128 * tm:128 * (tm + 1),
                                 C * hh:C * (hh + 1)],
                            zfin[:])

            for stage in (stage_load, stage_se, stage_xpose, stage_logits,
                          stage_attn, stage_gus):
                for s in range(BS):
                    stage(s)
            for s in range(BS):
                stage_g2(s)
                stage_down(s)

    nc.compile()
    return nc


def _host_prep(x, se_w1, se_b1, se_w2, se_b2, down_w, gus):
    bf = ml_dtypes.bfloat16
    x = np.ascontiguousarray(np.asarray(x, np.float32).reshape(B, C, HW))
    gusT = np.ascontiguousarray(np.asarray(gus, np.float32).reshape(HW, HW).T)
    # permute columns within each 128-block: evens first, then odds, so the
    # gaussian matmul's psum partitions give contiguous channel-halves
    perm = np.concatenate([np.arange(0, 128, 2), np.arange(1, 128, 2)])
    gusT = gusT.reshape(HW, 8, 128)[:, :, perm].reshape(HW, HW).astype(bf)
    dw = np.asarray(down_w, np.float32)
    dw1T = np.ascontiguousarray(dw[:, :C].T).astype(bf)           # [C, C]
    dw2 = dw[:, C:]                                               # [C, C]
    d2eT = np.zeros((HW, C), np.float32)
    d2oT = np.zeros((HW, C), np.float32)
    d2eT[0::2, :] = dw2.T
    d2oT[1::2, :] = dw2.T
    d2eT = d2eT.astype(bf)
    d2oT = d2oT.astype(bf)
    w1T = np.ascontiguousarray(np.asarray(se_w1, np.float32).T) / HW
    w2T = np.ascontiguousarray(np.asarray(se_w2, np.float32).T)
    b1 = np.asarray(se_b1, np.float32).reshape(32, 1)
    b2 = np.asarray(se_b2, np.float32).reshape(C, 1)
    ident = np.eye(128, dtype=np.float32)
    identb = np.eye(128, dtype=np.float32).astype(bf)
    ones1 = np.ones((1, 128), np.float32)
    oneh = np.zeros((9, 128, 9), np.float32)
    for k in range(9):
        oneh[k, :, k] = 1.0 / C
    oneh = oneh.astype(bf)

    # masks over the logits tiles: [8 tiles, 128 rows, 9 offsets]
    # x-edges: x==0 kills dx=-1 cols {0,3,6}; x==31 kills dx=+1 cols {2,5,8}
    r = np.arange(128)
    maskA_x = np.ones((128, 9), np.float32)
    maskA_x[r % 32 == 0, 0] = 0
    maskA_x[r % 32 == 0, 3] = 0
    maskA_x[r % 32 == 0, 6] = 0
    maskA_x[r % 32 == 31, 2] = 0
    maskA_x[r % 32 == 31, 5] = 0
    maskA_x[r % 32 == 31, 8] = 0
    maskL = np.broadcast_to(maskA_x, (8, 128, 9)).copy()
    # y-edges: first image row (tile 0, rows<32) kills dy=-1 cols {0,1,2};
    # last image row (tile 7, rows>=96) kills dy=+1 cols {6,7,8}
    maskL[0, :32, 0:3] = 0
    maskL[7, 96:, 6:9] = 0
    # attention export mask: x-edges only; y-edge entries are excluded by
    # the diagonal DMA ranges.
    maskA_t = maskA_x.astype(bf)

    atz = np.zeros((BS, HW, HW), bf)

    shared = {
        "gusT": gusT, "dw1T": dw1T, "d2eT": d2eT, "d2oT": d2oT,
        "w1T": w1T, "w2T": w2T,
        "b1": b1, "b2": b2, "ident": ident, "identb": identb,
        "ones1": ones1, "oneh": oneh,
        "maskL": maskL, "maskA": maskA_t, "atz": atz,
    }
    in_maps = []
    for i in range(NCORES):
        m = dict(shared)
        m["x2"] = np.ascontiguousarray(x[BS * i:BS * (i + 1)]).astype(bf)
        in_maps.append(m)
    return in_maps


def _ensure_ntff_hook():
    """Wire the missing antenv.axon_hooks shim so trace=True works."""
    import sys
    import types
    try:
        import antenv.axon_hooks  # noqa: F401
        return
    except ImportError:
        pass
    from trn_agent_boot.trn_boot import _ntff_profile_via_ctypes
    hook = _ntff_profile_via_ctypes("/opt/axon/libaxon_pjrt.so")
    mod = types.ModuleType("antenv.axon_hooks")
    mod.get_axon_ntff_profile_hook = lambda: hook
    mod.set_axon_ntff_profile_hook = lambda h: None
    sys.modules["antenv.axon_hooks"] = mod


def kernel(x, se_w1, se_b1, se_w2, se_b2, down_w, gus, _trace=False):
    from concourse import bass_utils

    if _trace:
        try:
            _ensure_ntff_hook()
        except Exception:
            _trace = False
    if "nc" not in _CACHE:
        _CACHE["nc"] = _build_nc()
    nc = _CACHE["nc"]
    in_maps = _host_prep(x, se_w1, se_b1, se_w2, se_b2, down_w, gus)
    res = bass_utils.run_bass_kernel_spmd(
        nc, in_maps, core_ids=list(range(NCORES)), trace=_trace)
    _CACHE["last_result"] = res
    outs = [res.results[i]["out2"] for i in range(NCORES)]
    out = np.concatenate(outs, axis=0).reshape(B, C, H, W)
    return out.astype(np.float32)
